# revision 1
# baseline (speedup 1.0000x reference)
"""Trainium2 Bass kernel for a GAT block (GATConv + LN + FFN + LN).

Self-contained: builds per-core shards on the host, compiles one SPMD Bass
program, runs it on 8 NeuronCores via run_bass_kernel_spmd, and reassembles
the full [50000, 128] output.

Per-core scheme (core c of 8, nodes permuted own-first per core):
  Phase A: ps = x_tile @ [W_fh | W@Asrc | W@Adst] for all 50176 (padded)
           nodes; [h_fh | a_src] rows (136 bf16) stored flat per-partition
           to DRAM (row r of the gather space lives at partition r%128,
           offset (r//128)*136) so stores are large contiguous descriptors;
           a_dst rows (8 bf16) stored to a [NBLK, 128, 8] block-major table.
           h_fh = h with columns permuted (f,h)-major so the phase-B message
           multiply broadcasts p over a middle dim (keeps DVE 2x mode).
  Phase B: edges with dst owned by the core (incl self-loops), grouped by
           128-node block, split lo/hi on the 32K int16 gather-index limit,
           padded per (block, stream) to 128-edge granules with a shared
           max-over-cores profile so all cores run one program.
           Per chunk: one dma_gather of [h|a_src] rows (elem_size=256 span,
           elem_step=136). The edge->dst one-hot S [e,d] and its transpose
           ST [d,e] are STATIC and shipped from the host as fp8.
           Per block: ad_blk [128,8] load; per granule adE = ST^T @ ad_blk
           (PE); eL = a_src + adE; p = exp(leaky(eL)) written over a_src in
           the gathered tile; msg = h*p in place; psum[block] += S^T @ [msg|p].
  Phase C: g = agg/denom (unpermuted back to (h,f) in the same op);
           u = LN(x + g); ff = relu(u@W1 + b1)@W2 + b2; z = LN(u + ff).
           LN rsqrt via exp(-0.5*ln(var+eps)) so the Act engine stays on a
           single function table (natural_log_exp_and_others) all kernel.
"""
import numpy as np
import ml_dtypes

N = 50000
NCORES = 8
OWN = 6272             # nodes per core (49 tiles of 128)
NP = OWN * NCORES      # padded node count
BLK = 128              # aggregation block == node tile
NBLK = OWN // BLK      # 49
NT = NP // 128         # 392 node tiles
GR = 128               # edges per granule
CHUNK = 4096           # edges per gather chunk
GPC = CHUNK // GR      # granules per chunk
LO_LIM = 1 << 15
H, F, D = 8, 16, 128
LN_EPS = 1e-5

bf16 = ml_dtypes.bfloat16
fp8 = ml_dtypes.float8_e4m3


def _wrap16(idx):
    L = idx.shape[0]
    w = idx.reshape(L // 16, 16).T.astype(np.int16)
    return np.tile(w, (8, 1))                      # [128, L/16]


def _bfr(x):
    return np.ascontiguousarray(x, dtype=np.float32).astype(bf16)


def _build_host_data(inputs):
    x = np.asarray(inputs["x"], np.float32)
    W = np.asarray(inputs["W_gat"], np.float32)
    att_src = np.asarray(inputs["att_src"], np.float32)
    att_dst = np.asarray(inputs["att_dst"], np.float32)
    ei = np.asarray(inputs["edge_index"])

    src = ei[0].astype(np.int64)
    dst = ei[1].astype(np.int64)
    loops = np.arange(N, dtype=np.int64)
    src = np.concatenate([src, loops])
    dst = np.concatenate([dst, loops])

    # per-core own-first permutation
    perms = []
    invs = []
    allp = np.arange(NP, dtype=np.int64)
    for c in range(NCORES):
        own = allp[OWN * c: OWN * (c + 1)]
        rest = np.concatenate([allp[: OWN * c], allp[OWN * (c + 1):]])
        perm = np.concatenate([own, rest])
        inv = np.empty(NP, dtype=np.int64)
        inv[perm] = np.arange(NP)
        perms.append(perm)
        invs.append(inv)

    # per (core, block, stream) counts on partition-major gather indices
    counts = np.zeros((NCORES, NBLK, 2), dtype=np.int64)
    core_edges = []
    for c in range(NCORES):
        m = (dst >= OWN * c) & (dst < min(OWN * (c + 1), N))
        r = invs[c][src[m]]                       # permuted gather row
        s_g = (r % 128) * NT + r // 128           # partition-major storage idx
        d_l = dst[m] - OWN * c                    # own-local dst row
        blk = d_l // BLK
        lo = s_g < LO_LIM
        core_edges.append((s_g, d_l, blk, lo))
        for b in range(NBLK):
            mb = blk == b
            counts[c, b, 0] = np.sum(mb & lo)
            counts[c, b, 1] = np.sum(mb & ~lo)

    g_prof = np.ceil(counts.max(axis=0) / GR).astype(np.int64)   # [NBLK, 2]
    L = [int(g_prof[:, s].sum()) * GR for s in range(2)]
    L_LO, L_HI = L

    dcols = np.arange(BLK, dtype=np.int64)
    per_core = []
    for c in range(NCORES):
        s_g, d_l, blk, lo = core_edges[c]
        streams = []
        for sidx in range(2):
            mm = lo if sidx == 0 else ~lo
            Ls = L[sidx]
            gidx = np.zeros(Ls, dtype=np.int64)
            dl = np.full(Ls, -1, dtype=np.int64)   # -1 => pad edge, S row zero
            pos = 0
            for b in range(NBLK):
                mb = (blk == b) & mm
                k = int(np.sum(mb))
                cap = int(g_prof[b, sidx]) * GR
                gidx[pos:pos + k] = s_g[mb] - (0 if sidx == 0 else LO_LIM)
                dl[pos:pos + k] = d_l[mb] % BLK
                pos += cap
            # static one-hots: S [e, g, d] and its transpose ST [d, g, e], fp8
            dl2 = dl.reshape(-1, GR).T             # [128 e, Lg]
            S = (dl2[:, :, None] == dcols[None, None, :])
            Sst = np.ascontiguousarray(S).astype(fp8)
            STst = np.ascontiguousarray(S.transpose(2, 1, 0)).astype(fp8)
            streams.append({
                "gidx16": _wrap16(gidx),
                "S": Sst,
                "ST": STst,
                "dl": np.ascontiguousarray(dl2.astype(np.float32).astype(bf16)),
            })
        per_core.append(streams)

    # weights: Wp = [W_fh | W@Asrc | W@Adst]  (f,h)-major h columns
    Wfh = np.ascontiguousarray(
        W.reshape(D, H, F).transpose(0, 2, 1).reshape(D, D))
    Asrc = np.zeros((D, H), np.float32)
    Adst = np.zeros((D, H), np.float32)
    for h in range(H):
        Asrc[h * F:(h + 1) * F, h] = att_src[h]
        Adst[h * F:(h + 1) * F, h] = att_dst[h]
    Wp = _bfr(np.concatenate([Wfh, W @ Asrc, W @ Adst], axis=1))  # [128,144]
    iota = _bfr(np.tile(np.arange(BLK, dtype=np.float32), (128, 1)))
    I128 = _bfr(np.eye(128, dtype=np.float32))

    xp = np.zeros((NP, D), np.float32)
    xp[:N] = x
    xT_per_core = []
    xo_per_core = []
    for c in range(NCORES):
        xTc = np.ascontiguousarray(xp[perms[c]].T.astype(bf16))  # [128, NP]
        xT_per_core.append(xTc)
        xo = xp[OWN * c: OWN * (c + 1)].reshape(NBLK, 128, D)
        xo_per_core.append(np.ascontiguousarray(
            xo.transpose(1, 0, 2).reshape(128, NBLK * D).astype(bf16)))

    host = {
        "g_prof": g_prof, "L_LO": L_LO, "L_HI": L_HI,
        "per_core": per_core, "xT": xT_per_core, "x_ownP": xo_per_core,
        "Wp": Wp, "iota": iota, "I128": I128,
        "W1": _bfr(np.asarray(inputs["w_ff1"], np.float32)),     # [128,256]
        "W2": _bfr(np.asarray(inputs["w_ff2"], np.float32)),     # [256,128]
        "b1col": np.ascontiguousarray(
            np.asarray(inputs["b_ff1"], np.float32).reshape(2, 128).T),
    }
    host["bias_gat"] = np.asarray(inputs["bias_gat"], np.float32)
    host["b_ff2"] = np.asarray(inputs["b_ff2"], np.float32)
    for nm in ("gamma1", "beta1", "gamma2", "beta2"):
        host[nm] = np.asarray(inputs[nm], np.float32)
    host["triv_gb1"] = bool(np.all(host["gamma1"] == 1) and np.all(host["beta1"] == 0))
    host["triv_gb2"] = bool(np.all(host["gamma2"] == 1) and np.all(host["beta2"] == 0))
    host["triv_bgat"] = bool(np.all(host["bias_gat"] == 0))
    host["triv_bff2"] = bool(np.all(host["b_ff2"] == 0))
    return host


def _build_program(host):
    import concourse.bacc as bacc
    import concourse.mybir as mybir
    import concourse.tile as tile
    from concourse.bass import AP

    fp32 = mybir.dt.float32
    bft = mybir.dt.bfloat16
    f8 = mybir.dt.float8e4
    i16 = mybir.dt.int16
    Alu = mybir.AluOpType
    Act = mybir.ActivationFunctionType

    g_prof = host["g_prof"]
    L_LO, L_HI = host["L_LO"], host["L_HI"]

    nc = bacc.Bacc("TRN2")

    # ---- DRAM tensors ----
    xT_d = nc.dram_tensor("xT", [128, NP], bft, kind="ExternalInput")
    xoP_d = nc.dram_tensor("x_ownP", [128, NBLK * D], bft, kind="ExternalInput")
    Wp_d = nc.dram_tensor("Wp", [128, 144], bft, kind="ExternalInput")
    iota_d = nc.dram_tensor("iota", [128, BLK], bft, kind="ExternalInput")
    I128_d = nc.dram_tensor("I128", [128, 128], bft, kind="ExternalInput")
    W1_d = nc.dram_tensor("W1", [128, 256], bft, kind="ExternalInput")
    W2_d = nc.dram_tensor("W2", [256, 128], bft, kind="ExternalInput")
    b1c_d = nc.dram_tensor("b1col", [128, 2], fp32, kind="ExternalInput")
    gl_d = {}
    if not host["triv_bgat"]:
        gl_d["bgat"] = nc.dram_tensor("bgat_r", [128, 128], fp32, kind="ExternalInput")
    if not host["triv_bff2"]:
        gl_d["bff2"] = nc.dram_tensor("bff2_r", [128, 128], fp32, kind="ExternalInput")
    if not host["triv_gb1"]:
        gl_d["g1"] = nc.dram_tensor("g1_r", [128, 128], fp32, kind="ExternalInput")
        gl_d["b1"] = nc.dram_tensor("b1_r", [128, 128], fp32, kind="ExternalInput")
    if not host["triv_gb2"]:
        gl_d["g2"] = nc.dram_tensor("g2_r", [128, 128], fp32, kind="ExternalInput")
        gl_d["b2"] = nc.dram_tensor("b2_r", [128, 128], fp32, kind="ExternalInput")

    st_d = []
    for sname, Ls in (("lo", L_LO), ("hi", L_HI)):
        st_d.append({
            "gidx": nc.dram_tensor(f"gidx_{sname}", [128, Ls // 16], i16,
                                   kind="ExternalInput"),
            "S": nc.dram_tensor(f"S_{sname}", [128, Ls // GR, 128], f8,
                                kind="ExternalInput"),
            "dl": nc.dram_tensor(f"dl_{sname}", [128, Ls // GR], bft,
                                 kind="ExternalInput"),
            "ST": nc.dram_tensor(f"ST_{sname}", [128, Ls // GR, 128], f8,
                                 kind="ExternalInput"),
            "L": Ls,
        })

    # gather space: row r at partition r%128, slot (r//128); 256-elem slots
    # (512B) because dma_gather requires the row stride % 256 bytes == 0;
    # only the first 136 elems [h | a_src] of each slot are written/used
    h_d = nc.dram_tensor("h_scratch", [128, NT * 256], bft, kind="Internal")
    ad_d = nc.dram_tensor("adst_scratch", [NBLK, 128, 8], bft, kind="Internal")
    z_d = nc.dram_tensor("z", [128, NBLK * D], fp32, kind="ExternalOutput")

    with tile.TileContext(nc) as tc:
        # ================= consts + preloads =================
        cpool = tc.alloc_tile_pool(name="consts", bufs=1)
        Wp_s = cpool.tile([128, 144], bft)
        nc.sync.dma_start(out=Wp_s[:], in_=Wp_d[:])
        I128_s = cpool.tile([128, 128], bft)
        nc.sync.dma_start(out=I128_s[:], in_=I128_d[:])
        W1_s = cpool.tile([128, 256], bft)
        nc.sync.dma_start(out=W1_s[:], in_=W1_d[:])
        W2_s = cpool.tile([256 // 2, 2, 128], bft)   # [128, 2, 128]
        nc.sync.dma_start(out=W2_s[:],
                          in_=W2_d[:].rearrange("(k h) f -> h k f", k=2))
        b1c_s = cpool.tile([128, 2], fp32)
        nc.sync.dma_start(out=b1c_s[:], in_=b1c_d[:])
        xo_s = cpool.tile([128, NBLK * D], bft)
        nc.sync.dma_start(out=xo_s[:], in_=xoP_d[:])
        gl_s = {}
        for k, dref in gl_d.items():
            gl_s[k] = cpool.tile([128, 128], fp32, tag=f"gl_{k}")
            nc.sync.dma_start(out=gl_s[k][:], in_=dref[:])
        iota_s = cpool.tile([128, BLK], bft)
        nc.sync.dma_start(out=iota_s[:], in_=iota_d[:])
        eps_s = cpool.tile([128, 1], fp32)
        nc.vector.memset(eps_s[:], LN_EPS)
        gix_s = []
        dl_s = []
        for s in range(2):
            gix = cpool.tile([128, st_d[s]["L"] // 16], i16, tag=f"gixA{s}")
            nc.sync.dma_start(out=gix[:], in_=st_d[s]["gidx"][:])
            gix_s.append(gix)
            dlt = cpool.tile([128, st_d[s]["L"] // GR], bft, tag=f"dlA{s}")
            nc.sync.dma_start(out=dlt[:], in_=st_d[s]["dl"][:])
            dl_s.append(dlt)

        # ================= phase A =================
        with tc.tile_pool(name="pA", bufs=4) as pA, \
             tc.tile_pool(name="psA", bufs=4, space="PSUM") as psA:
            GT = 3                                   # node tiles per psum bank
            XB = 24                                  # node tiles per x DMA
            xt = None
            for tg in range((NT + GT - 1) // GT):
                t0 = tg * GT
                ntl = min(GT, NT - t0)
                if t0 % XB == 0:
                    nxb = min(XB, NT - t0)
                    xt = pA.tile([128, XB * 128], bft, tag="xt")
                    nc.sync.dma_start(out=xt[:, :nxb * 128],
                                      in_=xT_d[:, t0 * 128:(t0 + nxb) * 128])
                ps = psA.tile([128, GT, 144], fp32, tag="psA")
                for j in range(ntl):
                    jo = (t0 % XB) + j
                    nc.tensor.matmul(ps[:, j, :],
                                     lhsT=xt[:, jo * 128:(jo + 1) * 128],
                                     rhs=Wp_s[:], start=True, stop=True)
                if tg % 3 == 0:
                    stage = pA.tile([128, 3 * GT, 144], bft, tag="stage")
                if tg % 2 == 0:
                    nc.scalar.activation(
                        out=stage[:, (tg % 3) * GT:(tg % 3) * GT + ntl, :],
                        in_=ps[:, :ntl, :], func=Act.Copy)
                else:
                    nc.vector.tensor_copy(
                        out=stage[:, (tg % 3) * GT:(tg % 3) * GT + ntl, :],
                        in_=ps[:, :ntl, :])
                s0 = (tg - 2) * GT
                nst = 2 * GT + ntl
                if tg % 3 == 2 or tg == (NT + GT - 1) // GT - 1:
                    if tg % 3 != 2:
                        s0 = (tg - tg % 3) * GT
                        nst = (tg % 3) * GT + ntl
                    nc.scalar.dma_start(
                        out=h_d[:, s0 * 256:(s0 + nst) * 256].rearrange(
                            "p (j d) -> p j d", d=256)[:, :, 0:136],
                        in_=stage[:, :nst, 0:136])
                    if s0 < NBLK:
                        na = min(nst, NBLK - s0)
                        nc.scalar.dma_start(
                            out=ad_d[s0:s0 + na, :, :].rearrange("j p d -> p j d"),
                            in_=stage[:, :na, 136:144])


        # ================= phases B + C =================
        hflat = h_d[:].rearrange("p (t d) -> (p t) d", d=256)
        h_lo = hflat[0:LO_LIM]
        h_hi = hflat[LO_LIM:NP]
        starts = np.zeros((NBLK, 2), dtype=np.int64)   # granule start per block
        for s in range(2):
            starts[1:, s] = np.cumsum(g_prof[:-1, s])

        pB = tc.alloc_tile_pool(name="pB", bufs=2)
        pBs = tc.alloc_tile_pool(name="pBsmall", bufs=4)
        psB = tc.alloc_tile_pool(name="psB", bufs=3, space="PSUM")
        psE = tc.alloc_tile_pool(name="psE", bufs=2, space="PSUM")
        pC = tc.alloc_tile_pool(name="pC", bufs=2)
        psC = tc.alloc_tile_pool(name="psC", bufs=1, space="PSUM")
        pD = tc.alloc_tile_pool(name="pD", bufs=1)

        # whole a_dst table (784B/partition) resident in SBUF
        adT = pD.tile([128, NBLK, 8], bft)
        nc.sync.dma_start(out=adT[:], in_=ad_d[:].rearrange("j p d -> p j d"))

        # block owning each granule, per stream (static)
        blk_of_g = []
        for s in range(2):
            arr = np.zeros(int(g_prof[:, s].sum()), dtype=np.int64)
            for b in range(NBLK):
                arr[int(starts[b, s]):int(starts[b, s] + g_prof[b, s])] = b
            blk_of_g.append(arr)

        chunk_tiles = [{}, {}]        # per stream: chunk idx -> tiles
        n_chunk = [0]

        def emit_chunk(s, k):
            if k in chunk_tiles[s]:
                return chunk_tiles[s][k]
            sd = st_d[s]
            ngr = min(GPC, sd["L"] // GR - k * GPC)   # granules in this chunk
            ne = ngr * GR
            ha = pB.tile([128, GPC, 256], bft, tag=f"ha{s}")
            nc.gpsimd.dma_gather(
                ha[:, :ngr, :], h_lo if s == 0 else h_hi,
                gix_s[s][:, k * (CHUNK // 16):k * (CHUNK // 16) + ne // 16],
                ne, ne, 256, single_packet=False)
            if n_chunk[0] % 3 == 0:
                S_t = pB.tile([128, GPC, 128], bft, tag=f"Sv{s}")
                iota_b = AP(iota_s[:].tensor, iota_s[:].offset,
                            [iota_s[:].ap[0], [0, ngr], iota_s[:].ap[1]])
                nc.vector.tensor_tensor(
                    out=S_t[:, :ngr, :],
                    in0=dl_s[s][:, k * GPC:k * GPC + ngr].to_broadcast(
                        [128, ngr, BLK]),
                    in1=iota_b, op=Alu.is_equal)
            else:
                S_t = pB.tile([128, GPC, 128], f8, tag=f"S{s}")
                nc.scalar.dma_start(out=S_t[:, :ngr, :],
                                    in_=sd["S"][:, k * GPC:k * GPC + ngr, :])
            n_chunk[0] += 1
            ST_t = pB.tile([128, GPC, 128], f8, tag=f"ST{s}")
            nc.sync.dma_start(out=ST_t[:, :ngr, :],
                              in_=sd["ST"][:, k * GPC:k * GPC + ngr, :])
            # adE[e, h] per granule = ST^T @ a_dst rows of the owning block
            adE = psE.tile([128, GPC, 8], fp32, tag="adE")
            for gi in range(ngr):
                b = int(blk_of_g[s][k * GPC + gi])
                nc.tensor.matmul(adE[:, gi, :], lhsT=ST_t[:, gi, :],
                                 rhs=adT[:, b, :], start=True, stop=True)
            # eL = a_src + adE ; p = exp(leaky(eL)) written over a_src
            eL = pBs.tile([128, GPC, 8], fp32, tag=f"eL{s}")
            nc.vector.tensor_tensor(out=eL[:, :ngr, :],
                                    in0=ha[:, :ngr, 128:136],
                                    in1=adE[:, :ngr, :], op=Alu.add)
            eL2 = pBs.tile([128, GPC, 8], bft, tag=f"eL2{s}")
            nc.vector.scalar_tensor_tensor(
                out=eL2[:, :ngr, :], in0=eL[:, :ngr, :], scalar=0.2,
                in1=eL[:, :ngr, :], op0=Alu.mult, op1=Alu.max)
            nc.scalar.activation(out=ha[:, :ngr, 128:136], in_=eL2[:, :ngr, :],
                                 func=Act.Exp)
            # msg = h * p in place (p broadcast over f: DVE 2x kept)
            pv = ha[:, :ngr, 128:136]
            pb = AP(pv.tensor, pv.offset,
                    [pv.ap[0], pv.ap[1], [0, F], pv.ap[2]])
            nc.vector.tensor_tensor(
                out=ha[:, :ngr, 0:128].rearrange("p g (f h) -> p g f h", h=H),
                in0=ha[:, :ngr, 0:128].rearrange("p g (f h) -> p g f h", h=H),
                in1=pb, op=Alu.mult)
            res = {"ha": ha, "S": S_t}
            chunk_tiles[s][k] = res
            return res

        for b0 in range(0, NBLK, 2):
            npair = min(2, NBLK - b0)
            gt = pC.tile([128, 2, 128], fp32, tag="gt")
            for q in range(npair):
                b = b0 + q
                ps_blk = psB.tile([128, 136], fp32, tag="blk")
                tot = int(g_prof[b, 0] + g_prof[b, 1])
                done = 0
                for s in range(2):
                    for gi in range(int(g_prof[b, s])):
                        gg = int(starts[b, s]) + gi
                        ct = emit_chunk(s, gg // GPC)
                        gl = gg % GPC
                        nc.tensor.matmul(ps_blk[:],
                                         lhsT=ct["S"][:, gl, :],
                                         rhs=ct["ha"][:, gl, 0:136],
                                         start=(done == 0),
                                         stop=(done == tot - 1))
                        done += 1
                # g = agg * (1/denom); unpermute (f,h)->(h,f) via the out AP
                rec = pBs.tile([128, 8], fp32, tag="rec")
                nc.vector.reciprocal(out=rec[:], in_=ps_blk[:, 128:136])
                rv = rec[:]
                rb = AP(rv.tensor, rv.offset, [rv.ap[0], [0, F], rv.ap[1]])
                nc.vector.tensor_tensor(
                    out=gt[:, q, :].rearrange("p (h f) -> p f h", f=F),
                    in0=ps_blk[:, 0:128].rearrange("p (f h) -> p f h", h=H),
                    in1=rb, op=Alu.mult)
                if not host["triv_bgat"]:
                    nc.vector.tensor_tensor(out=gt[:, q, :], in0=gt[:, q, :],
                                            in1=gl_s["bgat"][:], op=Alu.add)
            # ---- phase C for the pair ----
            t1 = pC.tile([128, 2, 128], fp32, tag="t1")
            nc.gpsimd.tensor_tensor(
                out=t1[:, :npair, :],
                in0=xo_s[:, b0 * 128:(b0 + npair) * 128].rearrange(
                    "p (q d) -> p q d", d=128),
                in1=gt[:, :npair, :], op=Alu.add)

            def layer_norm2(tin, g_key, b_key, triv, tagp, out=None, otile=None):
                # tin: [128, npair, 128]; writes normalized rows to out slices
                bst = pBs.tile([128, 2, 6], fp32, tag=f"bst{tagp}")
                mv = pBs.tile([128, 2, 2], fp32, tag=f"mv{tagp}")
                for q in range(npair):
                    nc.vector.bn_stats(out=bst[:, q, :], in_=tin[:, q, :])
                    nc.vector.bn_aggr(out=mv[:, q, :], in_=bst[:, q, :])
                rstd = pBs.tile([128, 2, 1], fp32, tag=f"rstd{tagp}")
                nc.scalar.activation(out=rstd[:, :npair, :],
                                     in_=mv[:, :npair, 1:2],
                                     func=Act.Sqrt, bias=eps_s[:])
                nc.vector.reciprocal(out=rstd[:, :npair, :],
                                     in_=rstd[:, :npair, :])
                if otile is None:
                    otile = pC.tile([128, 2, 128], fp32, tag=f"ln{tagp}")
                    out = otile[:]
                for q in range(npair):
                    nc.vector.tensor_scalar(out=out[:, q, :], in0=tin[:, q, :],
                                            scalar1=mv[:, q, 0:1],
                                            op0=Alu.subtract,
                                            scalar2=rstd[:, q, :], op1=Alu.mult)
                    if not triv:
                        nc.vector.tensor_tensor(out=out[:, q, :], in0=out[:, q, :],
                                                in1=gl_s[g_key][:], op=Alu.mult)
                        nc.vector.tensor_tensor(out=out[:, q, :], in0=out[:, q, :],
                                                in1=gl_s[b_key][:], op=Alu.add)
                return out

            u = layer_norm2(t1[:, :npair, :], "g1", "b1", host["triv_gb1"], "1")
            u_bf = pC.tile([128, 2, 128], bft, tag="ubf")
            nc.scalar.activation(out=u_bf[:, :npair, :], in_=u[:, :npair, :],
                                 func=Act.Copy)
            uT_ps = psC.tile([128, 2, 128], bft, tag="uT")
            for q in range(npair):
                nc.tensor.transpose(uT_ps[:, q, :], in_=u_bf[:, q, :],
                                    identity=I128_s[:])
            uT = pC.tile([128, 2, 128], bft, tag="uTs")
            nc.scalar.activation(out=uT[:, :npair, :], in_=uT_ps[:, :npair, :],
                                 func=Act.Copy)
            f1ps = psC.tile([128, 2, 2, 128], fp32, tag="f1")
            for q in range(npair):
                for j in range(2):
                    nc.tensor.matmul(f1ps[:, q, j, :],
                                     lhsT=W1_s[:, j * 128:(j + 1) * 128],
                                     rhs=uT[:, q, :], start=True, stop=True)
            r1 = pC.tile([128, 2, 2, 128], bft, tag="r1")
            for j in range(2):
                nc.scalar.activation(out=r1[:, :npair, j, :],
                                     in_=f1ps[:, :npair, j, :],
                                     func=Act.Relu, bias=b1c_s[:, j:j + 1])
            zps = psC.tile([128, 2, 128], fp32, tag="zp")
            for q in range(npair):
                for j in range(2):
                    nc.tensor.matmul(zps[:, q, :], lhsT=r1[:, q, j, :],
                                     rhs=W2_s[:, j, :],
                                     start=(j == 0), stop=(j == 1))
            t2 = pC.tile([128, 2, 128], fp32, tag="t2")
            nc.vector.tensor_tensor(out=t2[:, :npair, :], in0=u[:, :npair, :],
                                    in1=zps[:, :npair, :], op=Alu.add)
            if not host["triv_bff2"]:
                for q in range(npair):
                    nc.vector.tensor_tensor(out=t2[:, q, :], in0=t2[:, q, :],
                                            in1=gl_s["bff2"][:], op=Alu.add)
            if b0 % 8 == 0:
                z4 = pC.tile([128, 8, 128], fp32, tag="z4")
            layer_norm2(t2[:, :npair, :], "g2", "b2", host["triv_gb2"], "2",
                        out=z4[:, (b0 % 8):(b0 % 8) + npair, :], otile=z4)
            if (b0 + npair) % 8 == 0 or b0 + npair == NBLK:
                zb0 = ((b0 + npair - 1) // 8) * 8
                nc.scalar.dma_start(
                    out=z_d[:, zb0 * 128:(b0 + npair) * 128],
                    in_=z4[:, :b0 + npair - zb0, :])

        for p in (pD, psC, pC, psE, psB, pBs, pB):
            p.release()
        cpool.release()

    nc.compile()
    return nc


def kernel(**inputs):
    from concourse.bass_utils import run_bass_kernel_spmd
    import os

    host = _build_host_data(inputs)
    nc = _build_program(host)

    in_maps = []
    for c in range(NCORES):
        m = {
            "xT": host["xT"][c],
            "x_ownP": host["x_ownP"][c],
            "Wp": host["Wp"], "iota": host["iota"], "I128": host["I128"],
            "W1": host["W1"], "W2": host["W2"], "b1col": host["b1col"],
        }
        if not host["triv_bgat"]:
            m["bgat_r"] = np.tile(host["bias_gat"].reshape(1, -1), (128, 1))
        if not host["triv_bff2"]:
            m["bff2_r"] = np.tile(host["b_ff2"].reshape(1, -1), (128, 1))
        if not host["triv_gb1"]:
            m["g1_r"] = np.tile(host["gamma1"].reshape(1, -1), (128, 1))
            m["b1_r"] = np.tile(host["beta1"].reshape(1, -1), (128, 1))
        if not host["triv_gb2"]:
            m["g2_r"] = np.tile(host["gamma2"].reshape(1, -1), (128, 1))
            m["b2_r"] = np.tile(host["beta2"].reshape(1, -1), (128, 1))
        for s, sname in ((0, "lo"), (1, "hi")):
            sd = host["per_core"][c][s]
            m[f"gidx_{sname}"] = sd["gidx16"]
            m[f"S_{sname}"] = sd["S"]
            m[f"dl_{sname}"] = sd["dl"]
            m[f"ST_{sname}"] = sd["ST"]
        in_maps.append(m)

    trace = bool(int(os.environ.get("GAT_TRACE", "0")))
    res = run_bass_kernel_spmd(nc, in_maps, core_ids=list(range(NCORES)),
                               trace=trace)
    if trace and res.exec_time_ns:
        print(f"HW exec time: {res.exec_time_ns} ns")
    if bool(int(os.environ.get("GAT_TIME", "0"))):
        try:
            from concourse.timeline_sim import TimelineSim
            ts = TimelineSim(nc)
            dur = ts.simulate()
            print(f"HW exec time: {dur:.0f} ns (cost-model timeline estimate)")
        except Exception as e:
            print("timeline sim failed:", e)

    out = np.zeros((N, D), np.float32)
    for c in range(NCORES):
        lo_n = OWN * c
        hi_n = min(OWN * (c + 1), N)
        zc = res.results[c]["z"].reshape(128, NBLK, D).transpose(1, 0, 2)
        out[lo_n:hi_n] = zc.reshape(OWN, D)[: hi_n - lo_n]
    return out



# revision 21
# speedup vs baseline: 1.0679x; 1.0679x over previous
"""Trainium2 Bass kernel for a GAT block (GATConv + LN + FFN + LN).

Self-contained: builds per-core shards on the host, compiles one SPMD Bass
program, runs it on 8 NeuronCores via run_bass_kernel_spmd, reassembles the
full [50000, 128] output.

Scheme (v2 — slot-scheduled, DMA-minimized):
  Nodes are assigned to (core, 32-node block) by balanced bin-packing on
  in-degree so per-block edge counts are nearly equal across cores; the
  SPMD-shared per-(block,stream) edge capacities then waste ~1-5% instead of
  the granule-padded ~12%.
  Phase A: per-core xT (fp8, rows trimmed to the core's used sources) times
  [W_fh | W@Asrc | W@Adst] (bf16) -> [h_fh | a_src] rows (136 bf16) stored to
  a 256-elem-slot DRAM gather table; a_dst of own nodes captured directly
  into SBUF (adT).
  Phase B: edges with dst owned by the core, laid out dst-block-major in two
  streams (lo/hi on the 32K int16 gather-index limit), block capacities
  shared across cores at EDGE granularity; 128-edge granules may straddle
  blocks: each (granule x block) intersection is a "slot".  Per 4096-edge
  chunk: dma_gather of [h|a_src] rows; S one-hot [e,32d] generated on DVE at
  2x via a materialized iota table; ST4 (transposed one-hot, 4 blocks
  class-packed on the 128 partitions) shipped as a static fp8 table; per
  slot adE = ST4^T @ adT rows; p = exp(leaky(a_src + adE)); msg = h*p (DVE
  2x, (f,h)-major broadcast); aggregation psum[tile][32*(b%4)..] += S^T @
  [msg|p] packs 4 blocks per psum bank.
  Phase C: per 128-node bank: g = agg/denom (unpermuted to (h,f));
  u = LN(x+g) with rsqrt via exp(-0.5*ln(var+eps)) (keeps Act on one
  function table); ff = relu(u@W1)@W2; z = LN(u+ff) stored bf16.
"""
import numpy as np
import ml_dtypes

N = 50000
NCORES = 8
OWN = 6272              # nodes per core (49 tiles of 128)
NP = OWN * NCORES       # padded node count (50176)
NBT = OWN // 128        # 49 node tiles per core
BLK = 32                # aggregation block
NBLK = OWN // BLK       # 196 blocks per core
NCLS = 128 // BLK       # 4 blocks class-packed per 128 partitions
H, F, D = 8, 16, 128
LN_EPS = 1e-5
GR = 128                # edges per granule
CHUNK = 4096            # edges per gather chunk
GPC = CHUNK // GR       # 32 granules per chunk
LO_LIM = 1 << 15

bf16 = ml_dtypes.bfloat16
fp8 = ml_dtypes.float8_e4m3


def _wrap16(idx):
    L = idx.shape[0]
    w = idx.reshape(L // 16, 16).T.astype(np.int16)
    return np.tile(w, (8, 1))                      # [128, L/16]


def _bfr(x):
    return np.ascontiguousarray(x, dtype=np.float32).astype(bf16)


def _balanced_blocks(deg):
    """Assign NP nodes to NCORES*NBLK blocks of BLK nodes with ~equal degree
    sums; blocks ranked by sum and dealt 8-at-a-time to the same position on
    each core so the cross-core max at each position is tight."""
    nblocks = NCORES * NBLK
    order = np.argsort(-deg, kind="stable")
    sums = np.zeros(nblocks)
    fill = np.zeros(nblocks, dtype=np.int64)
    members = np.empty((nblocks, BLK), dtype=np.int64)
    import heapq
    heap = [(0.0, i) for i in range(nblocks)]
    heapq.heapify(heap)
    for n in order:
        while True:
            s, i = heapq.heappop(heap)
            if fill[i] < BLK:
                break
        members[i, fill[i]] = n
        fill[i] += 1
        sums[i] += deg[n]
        if fill[i] < BLK:
            heapq.heappush(heap, (sums[i], i))
    rank = np.argsort(-sums, kind="stable")
    # position p on core c gets block rank[8p + c]
    own_nodes = np.empty((NCORES, OWN), dtype=np.int64)
    for p in range(NBLK):
        for c in range(NCORES):
            own_nodes[c, p * BLK:(p + 1) * BLK] = members[rank[8 * p + c]]
    return own_nodes


def _build_host_data(inputs):
    x = np.asarray(inputs["x"], np.float32)
    W = np.asarray(inputs["W_gat"], np.float32)
    att_src = np.asarray(inputs["att_src"], np.float32)
    att_dst = np.asarray(inputs["att_dst"], np.float32)
    ei = np.asarray(inputs["edge_index"])

    src = np.concatenate([ei[0].astype(np.int64), np.arange(NP, dtype=np.int64)])
    dst = np.concatenate([ei[1].astype(np.int64), np.arange(NP, dtype=np.int64)])

    deg = np.bincount(dst, minlength=NP).astype(np.float64)
    own_nodes = _balanced_blocks(deg)

    core_of = np.empty(NP, dtype=np.int64)
    pos_of = np.empty(NP, dtype=np.int64)        # own-position within core
    for c in range(NCORES):
        core_of[own_nodes[c]] = c
        pos_of[own_nodes[c]] = np.arange(OWN)

    ecore = core_of[dst]
    # ---- per-core rows (trimmed, own-first) ----
    perms = []
    rows_of = []            # global node -> per-core row (only valid for used)
    n_rows = []
    core_e = []
    for c in range(NCORES):
        m = ecore == c
        es, ed = src[m], dst[m]
        core_e.append((es, ed))
        used = np.unique(es)
        own_mask = np.zeros(NP, dtype=bool)
        own_mask[own_nodes[c]] = True
        rest = used[~own_mask[used]]
        perm = np.concatenate([own_nodes[c], rest])
        perms.append(perm)
        r = np.full(NP, -1, dtype=np.int64)
        r[perm] = np.arange(len(perm))
        rows_of.append(r)
        n_rows.append(len(perm))
    NT = (max(n_rows) + 127) // 128
    NR = NT * 128

    # ---- per-core edge -> (block pos, dl, stream, gather idx) ----
    ed_all = []
    for c in range(NCORES):
        es, ed = core_e[c]
        p = pos_of[ed]
        blk = p // BLK
        dl = p % BLK
        r = rows_of[c][es]
        s_g = (r % 128) * NT + r // 128
        lo = s_g < LO_LIM
        ed_all.append((s_g, blk, dl, lo))

    # ---- shared capacities ----
    cnt = np.zeros((NCORES, NBLK, 2), dtype=np.int64)
    for c in range(NCORES):
        s_g, blk, dl, lo = ed_all[c]
        for sidx in range(2):
            mm = lo if sidx == 0 else ~lo
            cnt[c, :, sidx] = np.bincount(blk[mm], minlength=NBLK)
    cap = cnt.max(axis=0)                         # [NBLK, 2]
    start = np.zeros((NBLK, 2), dtype=np.int64)
    L = [0, 0]
    for sidx in range(2):
        start[:, sidx] = np.concatenate([[0], np.cumsum(cap[:, sidx])[:-1]])
        L[sidx] = int(cap[:, sidx].sum())
        L[sidx] = ((L[sidx] + GR - 1) // GR) * GR
    L_LO, L_HI = L
    pad_edges = L_LO + L_HI - int(cnt.sum() / NCORES)

    # ---- shared slot schedule ----
    # slot = (stream, granule, block); ordered by stream-position
    slots = [[], []]        # per stream: list of dict
    nchunk = [0, 0]
    SLM = 0                 # max slots per chunk
    SGM = 0                 # max sg rows per chunk
    sg_off = {}             # (s, k) -> (offset into ST4 table, nsg)
    slot_of = [[], []]
    chunk_slot0 = [[], []]
    sg_total = 0
    for sidx in range(2):
        nchunk[sidx] = (L[sidx] + CHUNK - 1) // CHUNK
        bnd = []            # (gpos, block) slot starts
        for b in range(NBLK):
            s0, s1 = start[b, sidx], start[b, sidx] + cap[b, sidx]
            g0, g1 = s0 // GR, (s1 - 1) // GR if s1 > s0 else s0 // GR
            for g in range(g0, g1 + 1):
                lo_e = max(s0, g * GR)
                hi_e = min(s1, (g + 1) * GR)
                if hi_e > lo_e:
                    slots[sidx].append({"g": g, "b": b, "e0": lo_e, "e1": hi_e})
        # assign slots to chunks; sg == chunk-local slot index
        for k in range(nchunk[sidx]):
            ch_slots = [i for i, sl in enumerate(slots[sidx])
                        if sl["g"] // GPC == k]
            chunk_slot0[sidx].append(ch_slots[0] if ch_slots else 0)
            SLM = max(SLM, len(ch_slots))
            for i in ch_slots:
                sl = slots[sidx][i]
                sl["sg"] = i - ch_slots[0]
                sl["sl_loc"] = i - ch_slots[0]
            nsg = len(ch_slots)
            sg_off[(sidx, k)] = (sg_total, nsg)
            sg_total += nsg
            SGM = max(SGM, nsg)

    # ---- schedules ----
    # adE: per (s, k): list of (g_loc, sg, b, start, stop)
    ade_sched = {}
    for sidx in range(2):
        for k in range(nchunk[sidx]):
            ents = []
            per_g = {}
            for i, sl in enumerate(slots[sidx]):
                if sl["g"] // GPC != k:
                    continue
                per_g.setdefault(sl["g"], []).append(sl)
            for g, sls in sorted(per_g.items()):
                for ii, sl in enumerate(sls):
                    ents.append((g % GPC, sl["sg"], sl["b"],
                                 ii == 0, ii == len(sls) - 1))
            ade_sched[(sidx, k)] = ents
    # aggregation: per block: list of (s, k, sl_loc, g_loc, first, last)
    agg_sched = [[] for _ in range(NBLK)]
    for sidx in range(2):
        for i, sl in enumerate(slots[sidx]):
            k = sl["g"] // GPC
            agg_sched[sl["b"]].append((sidx, k, sl["sl_loc"], sl["g"] % GPC))
    for b in range(NBLK):
        n = len(agg_sched[b])
        agg_sched[b] = [(s, k, sloc, gloc, i == 0, i == n - 1)
                        for i, (s, k, sloc, gloc) in enumerate(agg_sched[b])]

    # ---- per-core stream tables ----
    per_core = []
    for c in range(NCORES):
        s_g, blk, dl, lo = ed_all[c]
        streams = []
        for sidx in range(2):
            mm = lo if sidx == 0 else ~lo
            Ls = L[sidx]
            gidx = np.zeros(Ls, dtype=np.int64)
            dlv = np.full(Ls, -1, dtype=np.int64)
            bv = np.full(Ls, -1, dtype=np.int64)
            sgs = s_g[mm] - (0 if sidx == 0 else LO_LIM)
            blks = blk[mm]
            dls = dl[mm]
            o = np.argsort(blks, kind="stable")
            sgs, blks, dls = sgs[o], blks[o], dls[o]
            p0 = 0
            for b in range(NBLK):
                kk = int(cnt[c, b, sidx])
                s0 = int(start[b, sidx])
                gidx[s0:s0 + kk] = sgs[p0:p0 + kk]
                dlv[s0:s0 + kk] = dls[p0:p0 + kk]
                bv[s0:s0 + kk] = b
                p0 += kk
            # dl table [128, nslots]: dl of edge (g*128+p) if block matches
            nsl = len(slots[sidx])
            dlt = np.full((128, nsl), -1.0, dtype=np.float32)
            streams.append({
                "gidx16": _wrap16(gidx),
                "dlv": dlv, "bv": bv, "dlt": dlt, "nsl": nsl,
            })
        per_core.append(streams)

    # fill dl tables + ST4 tables
    for c in range(NCORES):
        for sidx in range(2):
            sd = per_core[c][sidx]
            dlv, bv = sd["dlv"], sd["bv"]
            dlt = sd["dlt"]
            for i, sl in enumerate(slots[sidx]):
                g, b = sl["g"], sl["b"]
                seg_dl = dlv[g * GR:(g + 1) * GR]
                seg_b = bv[g * GR:(g + 1) * GR]
                col = np.where(seg_b == b, seg_dl, -1.0)
                if len(col) < GR:
                    col = np.concatenate([col, -np.ones(GR - len(col))])
                dlt[:, i] = col
            sd["dl_bf"] = np.ascontiguousarray(dlt.astype(bf16))
    # ST32 per core: [32, sg_total, 128] fp8 (one slot per sg, d on 0..31)
    for c in range(NCORES):
        st4 = np.zeros((32, sg_total, 128), dtype=np.float32)
        for sidx in range(2):
            sd = per_core[c][sidx]
            dlv, bv = sd["dlv"], sd["bv"]
            for i, sl in enumerate(slots[sidx]):
                k = sl["g"] // GPC
                off, _ = sg_off[(sidx, k)]
                sg = sl["sg"]
                g, b = sl["g"], sl["b"]
                seg_dl = dlv[g * GR:(g + 1) * GR]
                seg_b = bv[g * GR:(g + 1) * GR]
                sel = (seg_b == b) & (seg_dl >= 0)
                ee = np.nonzero(sel)[0]
                st4[seg_dl[ee].astype(np.int64), off + sg, ee] = 1.0
        per_core[c] = {"streams": per_core[c],
                       "st4": np.ascontiguousarray(st4.astype(fp8))}

    # ---- weights ----
    Wfh = np.ascontiguousarray(
        W.reshape(D, H, F).transpose(0, 2, 1).reshape(D, D))
    Asrc = np.zeros((D, H), np.float32)
    Adst = np.zeros((D, H), np.float32)
    for h in range(H):
        Asrc[h * F:(h + 1) * F, h] = att_src[h]
        Adst[h * F:(h + 1) * F, h] = att_dst[h]
    Wp = _bfr(np.concatenate([Wfh, W @ Asrc, W @ Adst], axis=1))  # [128,144]
    I128 = _bfr(np.eye(128, dtype=np.float32))
    # iota2[p, d, sl] = d
    iota2 = _bfr(np.tile(np.arange(BLK, dtype=np.float32)[None, :, None],
                         (128, 1, SLM)))

    xp = np.zeros((NP, D), np.float32)
    xp[:N] = x
    xT_per_core = []
    xo_per_core = []
    for c in range(NCORES):
        xr = np.zeros((NR, D), np.float32)
        xr[:n_rows[c]] = xp[perms[c]]
        xT_per_core.append(np.ascontiguousarray(xr.T.astype(fp8)))  # [128,NR]
        xo = xp[own_nodes[c]].reshape(NBT, 128, D)
        xo_per_core.append(np.ascontiguousarray(
            xo.transpose(1, 0, 2).reshape(128, NBT * D).astype(bf16)))

    host = {
        "NT": NT, "L_LO": L_LO, "L_HI": L_HI, "SLM": SLM, "SGM": SGM,
        "sg_total": sg_total, "nchunk": nchunk, "slots": slots,
        "sg_off": sg_off, "ade_sched": ade_sched, "agg_sched": agg_sched,
        "chunk_slot0": chunk_slot0, "pad_edges": pad_edges,
        "per_core": per_core, "xT": xT_per_core, "x_ownP": xo_per_core,
        "own_nodes": own_nodes,
        "Wp": Wp, "I128": I128, "iota2": iota2,
        "W1": _bfr(np.asarray(inputs["w_ff1"], np.float32)),     # [128,256]
        "W2": _bfr(np.asarray(inputs["w_ff2"], np.float32)),     # [256,128]
        "b1col": np.ascontiguousarray(
            np.asarray(inputs["b_ff1"], np.float32).reshape(2, 128).T),
    }
    host["bias_gat"] = np.asarray(inputs["bias_gat"], np.float32)
    host["b_ff2"] = np.asarray(inputs["b_ff2"], np.float32)
    for nm in ("gamma1", "beta1", "gamma2", "beta2"):
        host[nm] = np.asarray(inputs[nm], np.float32)
    host["triv_gb1"] = bool(np.all(host["gamma1"] == 1) and np.all(host["beta1"] == 0))
    host["triv_gb2"] = bool(np.all(host["gamma2"] == 1) and np.all(host["beta2"] == 0))
    host["triv_bgat"] = bool(np.all(host["bias_gat"] == 0))
    host["triv_bff2"] = bool(np.all(host["b_ff2"] == 0))
    return host


def _build_program(host):
    import concourse.bacc as bacc
    import concourse.mybir as mybir
    import concourse.tile as tile
    from concourse.bass import AP

    # The act-table insertion pass picks the first table containing each
    # function; with the default ordering Ln lands in natural_log (no exp)
    # and Exp in exp_and_others (no ln), so interleaved Ln/Exp reload the
    # table every time.  Put the superset table first so one table serves
    # exp+ln+copy+relu for the whole program.
    if not getattr(bacc, "_gat_table_reorder", False):
        _orig = bacc.get_activation_tables

        def _reordered(arch):
            t = dict(_orig(arch))
            pref = "natural_log_exp_and_others"
            if pref in t:
                t = {pref: t[pref],
                     **{k: v for k, v in t.items() if k != pref}}
            return t

        bacc.get_activation_tables = _reordered
        bacc._orig_gat_tables = _orig
        bacc._gat_table_reorder = True

    fp32 = mybir.dt.float32
    bft = mybir.dt.bfloat16
    f8 = mybir.dt.float8e4
    i16 = mybir.dt.int16
    Alu = mybir.AluOpType
    Act = mybir.ActivationFunctionType

    NT = host["NT"]
    NR = NT * 128
    L_LO, L_HI = host["L_LO"], host["L_HI"]
    SLM, SGM = host["SLM"], host["SGM"]
    nchunk = host["nchunk"]
    sg_off = host["sg_off"]
    ade_sched = host["ade_sched"]
    agg_sched = host["agg_sched"]
    chunk_slot0 = host["chunk_slot0"]
    slots = host["slots"]
    nsl = [len(slots[0]), len(slots[1])]

    nc = bacc.Bacc("TRN2")

    # ---- DRAM tensors ----
    xT_d = nc.dram_tensor("xT", [128, NR], f8, kind="ExternalInput")
    xoP_d = nc.dram_tensor("x_ownP", [128, NBT * D], bft, kind="ExternalInput")
    Wp_d = nc.dram_tensor("Wp", [128, 144], bft, kind="ExternalInput")
    iota2_d = nc.dram_tensor("iota2", [128, BLK * SLM], bft, kind="ExternalInput")
    I128_d = nc.dram_tensor("I128", [128, 128], bft, kind="ExternalInput")
    W1_d = nc.dram_tensor("W1", [128, 256], bft, kind="ExternalInput")
    W2_d = nc.dram_tensor("W2", [256, 128], bft, kind="ExternalInput")
    b1c_d = nc.dram_tensor("b1col", [128, 2], fp32, kind="ExternalInput")
    gl_d = {}
    if not host["triv_bgat"]:
        gl_d["bgat"] = nc.dram_tensor("bgat_r", [128, 128], fp32, kind="ExternalInput")
    if not host["triv_bff2"]:
        gl_d["bff2"] = nc.dram_tensor("bff2_r", [128, 128], fp32, kind="ExternalInput")
    if not host["triv_gb1"]:
        gl_d["g1"] = nc.dram_tensor("g1_r", [128, 128], fp32, kind="ExternalInput")
        gl_d["b1"] = nc.dram_tensor("b1_r", [128, 128], fp32, kind="ExternalInput")
    if not host["triv_gb2"]:
        gl_d["g2"] = nc.dram_tensor("g2_r", [128, 128], fp32, kind="ExternalInput")
        gl_d["b2"] = nc.dram_tensor("b2_r", [128, 128], fp32, kind="ExternalInput")

    st_d = []
    for sname, Ls, ns in (("lo", L_LO, nsl[0]), ("hi", L_HI, nsl[1])):
        st_d.append({
            "gidx": nc.dram_tensor(f"gidx_{sname}", [128, Ls // 16], i16,
                                   kind="ExternalInput"),
            "dl": nc.dram_tensor(f"dl_{sname}", [128, ns], bft,
                                 kind="ExternalInput"),
            "L": Ls,
        })
    ST4_d = nc.dram_tensor("ST4", [32, host["sg_total"], 128], f8,
                           kind="ExternalInput")
    ad_d = nc.dram_tensor("ad_scratch", [128, NBT * 8], bft, kind="Internal")

    h_d = nc.dram_tensor("h_scratch", [128, NT * 256], bft, kind="Internal")
    z_d = nc.dram_tensor("z", [128, NBT * D], bft, kind="ExternalOutput")

    with tile.TileContext(nc) as tc:
        # ================= consts + preloads =================
        cpool = tc.alloc_tile_pool(name="consts", bufs=1)
        Wp_s = cpool.tile([128, 144], bft)
        nc.sync.dma_start(out=Wp_s[:], in_=Wp_d[:])
        I128_s = cpool.tile([128, 128], bft)
        nc.sync.dma_start(out=I128_s[:], in_=I128_d[:])
        W1_s = cpool.tile([128, 256], bft)
        nc.sync.dma_start(out=W1_s[:], in_=W1_d[:])
        W2_s = cpool.tile([256 // 2, 2, 128], bft)   # [128, 2, 128]
        nc.sync.dma_start(out=W2_s[:],
                          in_=W2_d[:].rearrange("(k h) f -> h k f", k=2))
        b1c_s = cpool.tile([128, 2], fp32)
        nc.sync.dma_start(out=b1c_s[:], in_=b1c_d[:])
        xo_s = cpool.tile([128, NBT * D], bft)
        nc.sync.dma_start(out=xo_s[:], in_=xoP_d[:])
        gl_s = {}
        for k, dref in gl_d.items():
            gl_s[k] = cpool.tile([128, 128], fp32, tag=f"gl_{k}")
            nc.sync.dma_start(out=gl_s[k][:], in_=dref[:])
        iota2_s = cpool.tile([128, BLK, SLM], bft)
        nc.sync.dma_start(out=iota2_s[:],
                          in_=iota2_d[:].rearrange("p (d s) -> p d s", d=BLK))
        eps_s = cpool.tile([128, 1], fp32)
        nc.vector.memset(eps_s[:], LN_EPS)
        gix_s = []
        dl_s = []
        for s in range(2):
            gix = cpool.tile([128, st_d[s]["L"] // 16], i16, tag=f"gixA{s}")
            nc.sync.dma_start(out=gix[:], in_=st_d[s]["gidx"][:])
            gix_s.append(gix)
            dlt = cpool.tile([128, nsl[s]], bft, tag=f"dlA{s}")
            nc.sync.dma_start(out=dlt[:], in_=st_d[s]["dl"][:])
            dl_s.append(dlt)

        pD = tc.alloc_tile_pool(name="pD", bufs=1)
        adT32 = pD.tile([32, NBLK, 8], bft)        # a_dst block-major rows

        # ================= phase A =================
        with tc.tile_pool(name="pA", bufs=4) as pA, \
             tc.tile_pool(name="psA", bufs=4, space="PSUM") as psA:
            GT = 3
            XB = 24
            xt = None
            ngrp = (NT + GT - 1) // GT
            for tg in range(ngrp):
                t0 = tg * GT
                ntl = min(GT, NT - t0)
                if t0 % XB == 0:
                    nxb = min(XB, NT - t0)
                    xt = pA.tile([128, XB * 128], f8, tag="xt")
                    nc.sync.dma_start(out=xt[:, :nxb * 128],
                                      in_=xT_d[:, t0 * 128:(t0 + nxb) * 128])
                ps = psA.tile([128, GT, 144], fp32, tag="psA")
                for j in range(ntl):
                    jo = (t0 % XB) + j
                    nc.tensor.matmul(ps[:, j, :],
                                     lhsT=xt[:, jo * 128:(jo + 1) * 128],
                                     rhs=Wp_s[:], start=True, stop=True)
                if tg % 3 == 0:
                    stage = pA.tile([128, 3 * GT, 136], bft, tag="stage")
                if tg % 2 == 0:
                    nc.scalar.activation(
                        out=stage[:, (tg % 3) * GT:(tg % 3) * GT + ntl, :],
                        in_=ps[:, :ntl, 0:136], func=Act.Copy)
                else:
                    nc.vector.tensor_copy(
                        out=stage[:, (tg % 3) * GT:(tg % 3) * GT + ntl, :],
                        in_=ps[:, :ntl, 0:136])
                if t0 < NBT:                       # own a_dst -> DRAM
                    na = min(ntl, NBT - t0)
                    adst = pA.tile([128, GT, 8], bft, tag="adst")
                    nc.vector.tensor_copy(out=adst[:, :na, :],
                                          in_=ps[:, :na, 136:144])
                    nc.scalar.dma_start(
                        out=ad_d[:, t0 * 8:(t0 + na) * 8],
                        in_=adst[:, :na, :])
                s0 = (tg - 2) * GT
                nst = 2 * GT + ntl
                if tg % 3 == 2 or tg == ngrp - 1:
                    if tg % 3 != 2:
                        s0 = (tg - tg % 3) * GT
                        nst = (tg % 3) * GT + ntl
                    nc.scalar.dma_start(
                        out=h_d[:, s0 * 256:(s0 + nst) * 256].rearrange(
                            "p (j d) -> p j d", d=256)[:, :, 0:136],
                        in_=stage[:, :nst, :])

        # ================= phases B + C =================
        nc.sync.dma_start(
            out=adT32[:].rearrange("r (t q) h -> r t q h", q=4),
            in_=ad_d[:].rearrange("(q r) (t h) -> r t q h", q=4, h=8))
        hflat = h_d[:].rearrange("p (t d) -> (p t) d", d=256)
        h_lo = hflat[0:LO_LIM]
        h_hi = hflat[LO_LIM:NR]

        pB = tc.alloc_tile_pool(name="pB", bufs=2)
        pBt = tc.alloc_tile_pool(name="pBtab", bufs=3)
        pBs = tc.alloc_tile_pool(name="pBsmall", bufs=6)
        psB = tc.alloc_tile_pool(name="psB", bufs=3, space="PSUM")
        psE = tc.alloc_tile_pool(name="psE", bufs=2, space="PSUM")
        pC = tc.alloc_tile_pool(name="pC", bufs=2)
        psC = tc.alloc_tile_pool(name="psC", bufs=1, space="PSUM")

        chunk_tiles = [{}, {}]

        def emit_chunk(s, k):
            if k in chunk_tiles[s]:
                return chunk_tiles[s][k]
            sd = st_d[s]
            ngr = min(GPC, sd["L"] // GR - k * GPC)
            ne = ngr * GR
            ha = pB.tile([128, GPC, 256], bft, tag=f"ha{s}")
            nc.gpsimd.dma_gather(
                ha[:, :ngr, :], h_lo if s == 0 else h_hi,
                gix_s[s][:, k * (CHUNK // 16):k * (CHUNK // 16) + ne // 16],
                ne, ne, 256, single_packet=False)
            # S one-hot [e, d, slot] via is_equal at 2x
            sl0 = chunk_slot0[s][k]
            ns = sum(1 for sl in slots[s] if sl["g"] // GPC == k)
            S_t = pBt.tile([128, BLK, SLM], bft, tag=f"S{s}")
            if ns > 0:
                dv = dl_s[s][:, sl0:sl0 + ns]
                dl_b = AP(dv.tensor, dv.offset,
                          [dv.ap[0], [0, BLK], dv.ap[1]])
                nc.vector.tensor_tensor(out=S_t[:, :, :ns],
                                        in0=iota2_s[:, :, :ns],
                                        in1=dl_b, op=Alu.is_equal)
            # ST4 static fp8 table
            off, nsg = sg_off[(s, k)]
            ST_t = pBt.tile([32, SGM, 128], f8, tag=f"ST{s}")
            if nsg > 0:
                nc.sync.dma_start(out=ST_t[:, :nsg, :],
                                  in_=ST4_d[:, off:off + nsg, :])
            # adE per granule
            adE = psE.tile([128, GPC, 8], fp32, tag="adE")
            for (g_loc, sg, b, st, sp) in ade_sched[(s, k)]:
                nc.tensor.matmul(adE[:, g_loc, :],
                                 lhsT=ST_t[:, sg, :],
                                 rhs=adT32[:, b, :],
                                 start=st, stop=sp)
            # p = exp(leaky(a_src + adE)) over a_src in the gathered tile
            eL = pBs.tile([128, GPC, 8], fp32, tag=f"eL{s}")
            nc.vector.tensor_tensor(out=eL[:, :ngr, :],
                                    in0=ha[:, :ngr, 128:136],
                                    in1=adE[:, :ngr, :], op=Alu.add)
            eL2 = pBs.tile([128, GPC, 8], bft, tag=f"eL2{s}")
            nc.vector.scalar_tensor_tensor(
                out=eL2[:, :ngr, :], in0=eL[:, :ngr, :], scalar=0.2,
                in1=eL[:, :ngr, :], op0=Alu.mult, op1=Alu.max)
            nc.scalar.activation(out=ha[:, :ngr, 128:136], in_=eL2[:, :ngr, :],
                                 func=Act.Exp)
            pv = ha[:, :ngr, 128:136]
            pb = AP(pv.tensor, pv.offset,
                    [pv.ap[0], pv.ap[1], [0, F], pv.ap[2]])
            nc.vector.tensor_tensor(
                out=ha[:, :ngr, 0:128].rearrange("p g (f h) -> p g f h", h=H),
                in0=ha[:, :ngr, 0:128].rearrange("p g (f h) -> p g f h", h=H),
                in1=pb, op=Alu.mult)
            res = {"ha": ha, "S": S_t}
            chunk_tiles[s][k] = res
            return res

        def layer_norm2(tin, g_key, b_key, triv, tagp, npair, out=None,
                        otile=None):
            bst = pBs.tile([128, 2, 6], fp32, tag=f"bst{tagp}")
            for q in range(npair):
                nc.vector.bn_stats(out=bst[:, q, :], in_=tin[:, q, :])
            mv = pBs.tile([128, 2, 2], fp32, tag=f"mv{tagp}")
            for q in range(npair):
                nc.vector.bn_aggr(out=mv[:, q, :], in_=bst[:, q, :])
            # rstd = exp(-0.5 * ln(var + eps)); stays on the exp/ln table
            lnv = pBs.tile([128, 2, 1], fp32, tag=f"lnv{tagp}")
            nc.scalar.activation(out=lnv[:, :npair, :], in_=mv[:, :npair, 1:2],
                                 func=Act.Ln, bias=eps_s[:])
            rstd = pBs.tile([128, 2, 1], fp32, tag=f"rstd{tagp}")
            nc.scalar.activation(out=rstd[:, :npair, :], in_=lnv[:, :npair, :],
                                 func=Act.Exp, scale=-0.5)
            if otile is None:
                otile = pC.tile([128, 2, 128], fp32, tag=f"ln{tagp}")
                out = otile[:]
            for q in range(npair):
                nc.gpsimd.tensor_scalar(out=out[:, q, :], in0=tin[:, q, :],
                                        scalar1=mv[:, q, 0:1],
                                        op0=Alu.subtract,
                                        scalar2=rstd[:, q, :], op1=Alu.mult)
                if not triv:
                    nc.vector.tensor_tensor(out=out[:, q, :], in0=out[:, q, :],
                                            in1=gl_s[g_key][:], op=Alu.mult)
                    nc.vector.tensor_tensor(out=out[:, q, :], in0=out[:, q, :],
                                            in1=gl_s[b_key][:], op=Alu.add)
            return out

        banks = [None, None]
        for t in range(NBT):
            bank = psB.tile([128, 136], fp32, tag="bank")
            for q in range(NCLS):
                b = NCLS * t + q
                for (s, k, sloc, gloc, st, sp) in agg_sched[b]:
                    ct = emit_chunk(s, k)
                    nc.tensor.matmul(bank[BLK * q:BLK * (q + 1), :],
                                     lhsT=ct["S"][:, :, sloc],
                                     rhs=ct["ha"][:, gloc, 0:136],
                                     start=st, stop=sp,
                                     tile_position=(0, BLK * q))
            banks[t % 2] = bank
            if t % 2 == 0 and t != NBT - 1:
                continue
            # ---- phase C for the pair ----
            npair = 1 if t == NBT - 1 and t % 2 == 0 else 2
            t0 = t - npair + 1
            gt = pC.tile([128, 2, 128], fp32, tag="gt")
            rec = pBs.tile([128, 2, 8], fp32, tag="rec")
            for q in range(npair):
                nc.vector.reciprocal(out=rec[:, q, :],
                                     in_=banks[(t0 + q) % 2][:, 128:136])
            for q in range(npair):
                rv = rec[:, q, :]
                rb = AP(rv.tensor, rv.offset, [rv.ap[0], [0, F], rv.ap[1]])
                nc.vector.tensor_tensor(
                    out=gt[:, q, :].rearrange("p (h f) -> p f h", f=F),
                    in0=banks[(t0 + q) % 2][:, 0:128].rearrange(
                        "p (f h) -> p f h", h=H),
                    in1=rb, op=Alu.mult)
                if not host["triv_bgat"]:
                    nc.vector.tensor_tensor(out=gt[:, q, :], in0=gt[:, q, :],
                                            in1=gl_s["bgat"][:], op=Alu.add)
            t1 = pC.tile([128, 2, 128], fp32, tag="t1")
            nc.gpsimd.tensor_tensor(
                out=t1[:, :npair, :],
                in0=xo_s[:, t0 * 128:(t0 + npair) * 128].rearrange(
                    "p (q d) -> p q d", d=128),
                in1=gt[:, :npair, :], op=Alu.add)
            u = layer_norm2(t1[:, :npair, :], "g1", "b1", host["triv_gb1"],
                            "1", npair)
            u_bf = pC.tile([128, 2, 128], bft, tag="ubf")
            nc.scalar.activation(out=u_bf[:, :npair, :], in_=u[:, :npair, :],
                                 func=Act.Copy)
            uT_ps = psC.tile([128, 2, 128], bft, tag="uT")
            for q in range(npair):
                nc.tensor.transpose(uT_ps[:, q, :], in_=u_bf[:, q, :],
                                    identity=I128_s[:])
            uT = pC.tile([128, 2, 128], bft, tag="uTs")
            nc.scalar.activation(out=uT[:, :npair, :], in_=uT_ps[:, :npair, :],
                                 func=Act.Copy)
            f1ps = psC.tile([128, 2, 2, 128], fp32, tag="f1")
            for q in range(npair):
                for j in range(2):
                    nc.tensor.matmul(f1ps[:, q, j, :],
                                     lhsT=W1_s[:, j * 128:(j + 1) * 128],
                                     rhs=uT[:, q, :], start=True, stop=True)
            r1 = pC.tile([128, 2, 2, 128], bft, tag="r1")
            for j in range(2):
                nc.scalar.activation(out=r1[:, :npair, j, :],
                                     in_=f1ps[:, :npair, j, :],
                                     func=Act.Relu, bias=b1c_s[:, j:j + 1])
            zps = psC.tile([128, 2, 128], fp32, tag="zp")
            for q in range(npair):
                for j in range(2):
                    nc.tensor.matmul(zps[:, q, :], lhsT=r1[:, q, j, :],
                                     rhs=W2_s[:, j, :],
                                     start=(j == 0), stop=(j == 1))
            t2 = pC.tile([128, 2, 128], fp32, tag="t2")
            nc.vector.tensor_tensor(out=t2[:, :npair, :], in0=u[:, :npair, :],
                                    in1=zps[:, :npair, :], op=Alu.add)
            if not host["triv_bff2"]:
                for q in range(npair):
                    nc.vector.tensor_tensor(out=t2[:, q, :], in0=t2[:, q, :],
                                            in1=gl_s["bff2"][:], op=Alu.add)
            if t0 % 8 == 0:
                z4 = pC.tile([128, 8, 128], bft, tag="z4")
            layer_norm2(t2[:, :npair, :], "g2", "b2", host["triv_gb2"], "2",
                        npair, out=z4[:, (t0 % 8):(t0 % 8) + npair, :],
                        otile=z4)
            if (t0 + npair) % 8 == 0 or t0 + npair == NBT:
                zb0 = ((t0 + npair - 1) // 8) * 8
                nc.scalar.dma_start(
                    out=z_d[:, zb0 * 128:(t0 + npair) * 128],
                    in_=z4[:, :t0 + npair - zb0, :])

        for p in (psC, pC, psE, psB, pBs, pBt, pB, pD):
            p.release()
        cpool.release()

    nc.compile()
    # insert_act_table_loads indexed tables by position in the (reordered)
    # list we fed it, but walrus interprets act_func_set_id as the index
    # into act_info.json's original order — remap.
    my_names = list(bacc.get_activation_tables(nc.m.arch).keys())
    json_names = list(bacc._orig_gat_tables(nc.m.arch).keys())
    for b in nc.m.functions[0].blocks:
        for i in b.instructions:
            if type(i).__name__ == "InstLoadActFuncSet":
                i.act_func_set_id = json_names.index(my_names[i.act_func_set_id])
    return nc


def kernel(**inputs):
    from concourse.bass_utils import run_bass_kernel_spmd
    import os

    host = _build_host_data(inputs)
    if bool(int(os.environ.get("GAT_DEBUG", "0"))):
        print(f"NT={host['NT']} L_LO={host['L_LO']} L_HI={host['L_HI']} "
              f"slots={len(host['slots'][0])}+{len(host['slots'][1])} "
              f"sg_total={host['sg_total']} SLM={host['SLM']} SGM={host['SGM']} "
              f"pad_edges={host['pad_edges']}")
    nc = _build_program(host)

    in_maps = []
    for c in range(NCORES):
        m = {
            "xT": host["xT"][c],
            "x_ownP": host["x_ownP"][c],
            "Wp": host["Wp"], "I128": host["I128"],
            "iota2": host["iota2"].reshape(128, BLK * host["SLM"]),
            "W1": host["W1"], "W2": host["W2"], "b1col": host["b1col"],
            "ST4": host["per_core"][c]["st4"],
        }
        if not host["triv_bgat"]:
            m["bgat_r"] = np.tile(host["bias_gat"].reshape(1, -1), (128, 1))
        if not host["triv_bff2"]:
            m["bff2_r"] = np.tile(host["b_ff2"].reshape(1, -1), (128, 1))
        if not host["triv_gb1"]:
            m["g1_r"] = np.tile(host["gamma1"].reshape(1, -1), (128, 1))
            m["b1_r"] = np.tile(host["beta1"].reshape(1, -1), (128, 1))
        if not host["triv_gb2"]:
            m["g2_r"] = np.tile(host["gamma2"].reshape(1, -1), (128, 1))
            m["b2_r"] = np.tile(host["beta2"].reshape(1, -1), (128, 1))
        for s, sname in ((0, "lo"), (1, "hi")):
            sd = host["per_core"][c]["streams"][s]
            m[f"gidx_{sname}"] = sd["gidx16"]
            m[f"dl_{sname}"] = sd["dl_bf"]
        in_maps.append(m)

    res = run_bass_kernel_spmd(nc, in_maps, core_ids=list(range(NCORES)))
    if bool(int(os.environ.get("GAT_TIME", "0"))):
        try:
            from concourse.timeline_sim import TimelineSim
            ts = TimelineSim(nc)
            dur = ts.simulate()
            print(f"HW exec time: {dur:.0f} ns (cost-model timeline estimate)")
        except Exception as e:
            print("timeline sim failed:", e)

    out = np.zeros((N, D), np.float32)
    for c in range(NCORES):
        zc = res.results[c]["z"].astype(np.float32)     # [128, NBT*128]
        zc = zc.reshape(128, NBT, D).transpose(1, 0, 2).reshape(OWN, D)
        own = host["own_nodes"][c]
        real = own < N
        out[own[real]] = zc[real]
    return out


# revision 22
# speedup vs baseline: 1.0812x; 1.0125x over previous
"""Trainium2 Bass kernel for a GAT block (GATConv + LN + FFN + LN).

Self-contained: builds per-core shards on the host, compiles one SPMD Bass
program, runs it on 8 NeuronCores via run_bass_kernel_spmd, reassembles the
full [50000, 128] output.

Scheme (v2 — slot-scheduled, DMA-minimized):
  Nodes are assigned to (core, 32-node block) by balanced bin-packing on
  in-degree so per-block edge counts are nearly equal across cores; the
  SPMD-shared per-(block,stream) edge capacities then waste ~1-5% instead of
  the granule-padded ~12%.
  Phase A: per-core xT (fp8, rows trimmed to the core's used sources) times
  [W_fh | W@Asrc | W@Adst] (bf16) -> [h_fh | a_src] rows (136 bf16) stored to
  a 256-elem-slot DRAM gather table; a_dst of own nodes captured directly
  into SBUF (adT).
  Phase B: edges with dst owned by the core, laid out dst-block-major in two
  streams (lo/hi on the 32K int16 gather-index limit), block capacities
  shared across cores at EDGE granularity; 128-edge granules may straddle
  blocks: each (granule x block) intersection is a "slot".  Per 4096-edge
  chunk: dma_gather of [h|a_src] rows; S one-hot [e,32d] generated on DVE at
  2x via a materialized iota table; ST4 (transposed one-hot, 4 blocks
  class-packed on the 128 partitions) shipped as a static fp8 table; per
  slot adE = ST4^T @ adT rows; p = exp(leaky(a_src + adE)); msg = h*p (DVE
  2x, (f,h)-major broadcast); aggregation psum[tile][32*(b%4)..] += S^T @
  [msg|p] packs 4 blocks per psum bank.
  Phase C: per 128-node bank: g = agg/denom (unpermuted to (h,f));
  u = LN(x+g) with rsqrt via exp(-0.5*ln(var+eps)) (keeps Act on one
  function table); ff = relu(u@W1)@W2; z = LN(u+ff) stored bf16.
"""
import numpy as np
import ml_dtypes

N = 50000
NCORES = 8
OWN = 6272              # nodes per core (49 tiles of 128)
NP = OWN * NCORES       # padded node count (50176)
NBT = OWN // 128        # 49 node tiles per core
BLK = 32                # aggregation block
NBLK = OWN // BLK       # 196 blocks per core
NCLS = 128 // BLK       # 4 blocks class-packed per 128 partitions
H, F, D = 8, 16, 128
LN_EPS = 1e-5
GR = 128                # edges per granule
CHUNK = 4096            # edges per gather chunk
GPC = CHUNK // GR       # 32 granules per chunk
LO_LIM = 1 << 15

bf16 = ml_dtypes.bfloat16
fp8 = ml_dtypes.float8_e4m3


def _wrap16(idx):
    L = idx.shape[0]
    w = idx.reshape(L // 16, 16).T.astype(np.int16)
    return np.tile(w, (8, 1))                      # [128, L/16]


def _bfr(x):
    return np.ascontiguousarray(x, dtype=np.float32).astype(bf16)


def _balanced_blocks(deg):
    """Assign NP nodes to NCORES*NBLK blocks of BLK nodes with ~equal degree
    sums; blocks ranked by sum and dealt 8-at-a-time to the same position on
    each core so the cross-core max at each position is tight."""
    nblocks = NCORES * NBLK
    order = np.argsort(-deg, kind="stable")
    sums = np.zeros(nblocks)
    fill = np.zeros(nblocks, dtype=np.int64)
    members = np.empty((nblocks, BLK), dtype=np.int64)
    import heapq
    heap = [(0.0, i) for i in range(nblocks)]
    heapq.heapify(heap)
    for n in order:
        while True:
            s, i = heapq.heappop(heap)
            if fill[i] < BLK:
                break
        members[i, fill[i]] = n
        fill[i] += 1
        sums[i] += deg[n]
        if fill[i] < BLK:
            heapq.heappush(heap, (sums[i], i))
    rank = np.argsort(-sums, kind="stable")
    # position p on core c gets block rank[8p + c]
    own_nodes = np.empty((NCORES, OWN), dtype=np.int64)
    for p in range(NBLK):
        for c in range(NCORES):
            own_nodes[c, p * BLK:(p + 1) * BLK] = members[rank[8 * p + c]]
    return own_nodes


def _build_host_data(inputs):
    x = np.asarray(inputs["x"], np.float32)
    W = np.asarray(inputs["W_gat"], np.float32)
    att_src = np.asarray(inputs["att_src"], np.float32)
    att_dst = np.asarray(inputs["att_dst"], np.float32)
    ei = np.asarray(inputs["edge_index"])

    src = np.concatenate([ei[0].astype(np.int64), np.arange(NP, dtype=np.int64)])
    dst = np.concatenate([ei[1].astype(np.int64), np.arange(NP, dtype=np.int64)])

    deg = np.bincount(dst, minlength=NP).astype(np.float64)
    own_nodes = _balanced_blocks(deg)

    core_of = np.empty(NP, dtype=np.int64)
    pos_of = np.empty(NP, dtype=np.int64)        # own-position within core
    for c in range(NCORES):
        core_of[own_nodes[c]] = c
        pos_of[own_nodes[c]] = np.arange(OWN)

    ecore = core_of[dst]
    # ---- per-core rows (trimmed, own-first) ----
    perms = []
    rows_of = []            # global node -> per-core row (only valid for used)
    n_rows = []
    core_e = []
    for c in range(NCORES):
        m = ecore == c
        es, ed = src[m], dst[m]
        core_e.append((es, ed))
        used = np.unique(es)
        own_mask = np.zeros(NP, dtype=bool)
        own_mask[own_nodes[c]] = True
        rest = used[~own_mask[used]]
        perm = np.concatenate([own_nodes[c], rest])
        perms.append(perm)
        r = np.full(NP, -1, dtype=np.int64)
        r[perm] = np.arange(len(perm))
        rows_of.append(r)
        n_rows.append(len(perm))
    NT = (max(n_rows) + 127) // 128
    NR = NT * 128

    # ---- per-core edge -> (block pos, dl, stream, gather idx) ----
    ed_all = []
    for c in range(NCORES):
        es, ed = core_e[c]
        p = pos_of[ed]
        blk = p // BLK
        dl = p % BLK
        r = rows_of[c][es]
        s_g = (r % 128) * NT + r // 128
        lo = s_g < LO_LIM
        ed_all.append((s_g, blk, dl, lo))

    # ---- shared capacities ----
    cnt = np.zeros((NCORES, NBLK, 2), dtype=np.int64)
    for c in range(NCORES):
        s_g, blk, dl, lo = ed_all[c]
        for sidx in range(2):
            mm = lo if sidx == 0 else ~lo
            cnt[c, :, sidx] = np.bincount(blk[mm], minlength=NBLK)
    cap = cnt.max(axis=0)                         # [NBLK, 2]
    start = np.zeros((NBLK, 2), dtype=np.int64)
    L = [0, 0]
    for sidx in range(2):
        start[:, sidx] = np.concatenate([[0], np.cumsum(cap[:, sidx])[:-1]])
        L[sidx] = int(cap[:, sidx].sum())
        L[sidx] = ((L[sidx] + GR - 1) // GR) * GR
    L_LO, L_HI = L
    pad_edges = L_LO + L_HI - int(cnt.sum() / NCORES)

    # ---- shared slot schedule ----
    # slot = (stream, granule, block); ordered by stream-position
    slots = [[], []]        # per stream: list of dict
    nchunk = [0, 0]
    SLM = 0                 # max slots per chunk
    SGM = 0                 # max sg rows per chunk
    sg_off = {}             # (s, k) -> (offset into ST4 table, nsg)
    slot_of = [[], []]
    chunk_slot0 = [[], []]
    sg_total = 0
    for sidx in range(2):
        nchunk[sidx] = (L[sidx] + CHUNK - 1) // CHUNK
        bnd = []            # (gpos, block) slot starts
        for b in range(NBLK):
            s0, s1 = start[b, sidx], start[b, sidx] + cap[b, sidx]
            g0, g1 = s0 // GR, (s1 - 1) // GR if s1 > s0 else s0 // GR
            for g in range(g0, g1 + 1):
                lo_e = max(s0, g * GR)
                hi_e = min(s1, (g + 1) * GR)
                if hi_e > lo_e:
                    slots[sidx].append({"g": g, "b": b, "e0": lo_e, "e1": hi_e})
        # assign slots to chunks; sg == chunk-local slot index
        for k in range(nchunk[sidx]):
            ch_slots = [i for i, sl in enumerate(slots[sidx])
                        if sl["g"] // GPC == k]
            chunk_slot0[sidx].append(ch_slots[0] if ch_slots else 0)
            SLM = max(SLM, len(ch_slots))
            for i in ch_slots:
                sl = slots[sidx][i]
                sl["sg"] = i - ch_slots[0]
                sl["sl_loc"] = i - ch_slots[0]
            nsg = len(ch_slots)
            sg_off[(sidx, k)] = (sg_total, nsg)
            sg_total += nsg
            SGM = max(SGM, nsg)

    # ---- schedules ----
    # adE: per (s, k): list of (g_loc, sg, b, start, stop)
    ade_sched = {}
    for sidx in range(2):
        for k in range(nchunk[sidx]):
            ents = []
            per_g = {}
            for i, sl in enumerate(slots[sidx]):
                if sl["g"] // GPC != k:
                    continue
                per_g.setdefault(sl["g"], []).append(sl)
            for g, sls in sorted(per_g.items()):
                for ii, sl in enumerate(sls):
                    ents.append((g % GPC, sl["sg"], sl["b"],
                                 ii == 0, ii == len(sls) - 1))
            ade_sched[(sidx, k)] = ents
    # aggregation: per block: list of (s, k, sl_loc, g_loc, first, last)
    agg_sched = [[] for _ in range(NBLK)]
    for sidx in range(2):
        for i, sl in enumerate(slots[sidx]):
            k = sl["g"] // GPC
            agg_sched[sl["b"]].append((sidx, k, sl["sl_loc"], sl["g"] % GPC))
    for b in range(NBLK):
        n = len(agg_sched[b])
        agg_sched[b] = [(s, k, sloc, gloc, i == 0, i == n - 1)
                        for i, (s, k, sloc, gloc) in enumerate(agg_sched[b])]

    # ---- per-core stream tables ----
    per_core = []
    for c in range(NCORES):
        s_g, blk, dl, lo = ed_all[c]
        streams = []
        for sidx in range(2):
            mm = lo if sidx == 0 else ~lo
            Ls = L[sidx]
            gidx = np.zeros(Ls, dtype=np.int64)
            dlv = np.full(Ls, -1, dtype=np.int64)
            bv = np.full(Ls, -1, dtype=np.int64)
            sgs = s_g[mm] - (0 if sidx == 0 else LO_LIM)
            blks = blk[mm]
            dls = dl[mm]
            o = np.argsort(blks, kind="stable")
            sgs, blks, dls = sgs[o], blks[o], dls[o]
            p0 = 0
            for b in range(NBLK):
                kk = int(cnt[c, b, sidx])
                s0 = int(start[b, sidx])
                gidx[s0:s0 + kk] = sgs[p0:p0 + kk]
                dlv[s0:s0 + kk] = dls[p0:p0 + kk]
                bv[s0:s0 + kk] = b
                p0 += kk
            # dl table [128, nslots]: dl of edge (g*128+p) if block matches
            nsl = len(slots[sidx])
            dlt = np.full((128, nsl), -1.0, dtype=np.float32)
            streams.append({
                "gidx16": _wrap16(gidx),
                "dlv": dlv, "bv": bv, "dlt": dlt, "nsl": nsl,
            })
        per_core.append(streams)

    # fill dl tables + ST4 tables
    for c in range(NCORES):
        for sidx in range(2):
            sd = per_core[c][sidx]
            dlv, bv = sd["dlv"], sd["bv"]
            dlt = sd["dlt"]
            for i, sl in enumerate(slots[sidx]):
                g, b = sl["g"], sl["b"]
                seg_dl = dlv[g * GR:(g + 1) * GR]
                seg_b = bv[g * GR:(g + 1) * GR]
                col = np.where(seg_b == b, seg_dl, -1.0)
                if len(col) < GR:
                    col = np.concatenate([col, -np.ones(GR - len(col))])
                dlt[:, i] = col
            sd["dl_bf"] = np.ascontiguousarray(dlt.astype(bf16))
    # ST32 per core: [32, sg_total, 128] fp8 (one slot per sg, d on 0..31)
    for c in range(NCORES):
        st4 = np.zeros((32, sg_total, 128), dtype=np.float32)
        for sidx in range(2):
            sd = per_core[c][sidx]
            dlv, bv = sd["dlv"], sd["bv"]
            for i, sl in enumerate(slots[sidx]):
                k = sl["g"] // GPC
                off, _ = sg_off[(sidx, k)]
                sg = sl["sg"]
                g, b = sl["g"], sl["b"]
                seg_dl = dlv[g * GR:(g + 1) * GR]
                seg_b = bv[g * GR:(g + 1) * GR]
                sel = (seg_b == b) & (seg_dl >= 0)
                ee = np.nonzero(sel)[0]
                st4[seg_dl[ee].astype(np.int64), off + sg, ee] = 1.0
        per_core[c] = {"streams": per_core[c],
                       "st4": np.ascontiguousarray(st4.astype(fp8))}

    # ---- weights ----
    Wfh = np.ascontiguousarray(
        W.reshape(D, H, F).transpose(0, 2, 1).reshape(D, D))
    Asrc = np.zeros((D, H), np.float32)
    Adst = np.zeros((D, H), np.float32)
    for h in range(H):
        Asrc[h * F:(h + 1) * F, h] = att_src[h]
        Adst[h * F:(h + 1) * F, h] = att_dst[h]
    Wp = _bfr(np.concatenate([Wfh, W @ Asrc, W @ Adst], axis=1))  # [128,144]
    I128 = _bfr(np.eye(128, dtype=np.float32))
    # iota2[p, d, sl] = d
    iota2 = _bfr(np.tile(np.arange(BLK, dtype=np.float32)[None, :, None],
                         (128, 1, SLM)))

    xp = np.zeros((NP, D), np.float32)
    xp[:N] = x
    xT_per_core = []
    xo_per_core = []
    for c in range(NCORES):
        xr = np.zeros((NR, D), np.float32)
        xr[:n_rows[c]] = xp[perms[c]]
        xT_per_core.append(np.ascontiguousarray(xr.T.astype(fp8)))  # [128,NR]
        xo = xp[own_nodes[c]].reshape(NBT, 128, D)
        xo_per_core.append(np.ascontiguousarray(
            xo.transpose(1, 0, 2).reshape(128, NBT * D).astype(bf16)))

    host = {
        "NT": NT, "L_LO": L_LO, "L_HI": L_HI, "SLM": SLM, "SGM": SGM,
        "sg_total": sg_total, "nchunk": nchunk, "slots": slots,
        "sg_off": sg_off, "ade_sched": ade_sched, "agg_sched": agg_sched,
        "chunk_slot0": chunk_slot0, "pad_edges": pad_edges,
        "per_core": per_core, "xT": xT_per_core, "x_ownP": xo_per_core,
        "own_nodes": own_nodes,
        "Wp": Wp, "I128": I128, "iota2": iota2,
        "W1": _bfr(np.asarray(inputs["w_ff1"], np.float32)),     # [128,256]
        "W2": _bfr(np.asarray(inputs["w_ff2"], np.float32)),     # [256,128]
        "b1col": np.ascontiguousarray(
            np.asarray(inputs["b_ff1"], np.float32).reshape(2, 128).T),
    }
    host["bias_gat"] = np.asarray(inputs["bias_gat"], np.float32)
    host["b_ff2"] = np.asarray(inputs["b_ff2"], np.float32)
    for nm in ("gamma1", "beta1", "gamma2", "beta2"):
        host[nm] = np.asarray(inputs[nm], np.float32)
    host["triv_gb1"] = bool(np.all(host["gamma1"] == 1) and np.all(host["beta1"] == 0))
    host["triv_gb2"] = bool(np.all(host["gamma2"] == 1) and np.all(host["beta2"] == 0))
    host["triv_bgat"] = bool(np.all(host["bias_gat"] == 0))
    host["triv_bff2"] = bool(np.all(host["b_ff2"] == 0))
    return host


def _build_program(host):
    import concourse.bacc as bacc
    import concourse.mybir as mybir
    import concourse.tile as tile
    from concourse.bass import AP

    # The act-table insertion pass picks the first table containing each
    # function; with the default ordering Ln lands in natural_log (no exp)
    # and Exp in exp_and_others (no ln), so interleaved Ln/Exp reload the
    # table every time.  Put the superset table first so one table serves
    # exp+ln+copy+relu for the whole program.
    if not getattr(bacc, "_gat_table_reorder", False):
        _orig = bacc.get_activation_tables

        def _reordered(arch):
            t = dict(_orig(arch))
            pref = "natural_log_exp_and_others"
            if pref in t:
                t = {pref: t[pref],
                     **{k: v for k, v in t.items() if k != pref}}
            return t

        bacc.get_activation_tables = _reordered
        bacc._orig_gat_tables = _orig
        bacc._gat_table_reorder = True

    fp32 = mybir.dt.float32
    bft = mybir.dt.bfloat16
    f8 = mybir.dt.float8e4
    i16 = mybir.dt.int16
    Alu = mybir.AluOpType
    Act = mybir.ActivationFunctionType

    NT = host["NT"]
    NR = NT * 128
    L_LO, L_HI = host["L_LO"], host["L_HI"]
    SLM, SGM = host["SLM"], host["SGM"]
    nchunk = host["nchunk"]
    sg_off = host["sg_off"]
    ade_sched = host["ade_sched"]
    agg_sched = host["agg_sched"]
    chunk_slot0 = host["chunk_slot0"]
    slots = host["slots"]
    nsl = [len(slots[0]), len(slots[1])]

    nc = bacc.Bacc("TRN2")

    # ---- DRAM tensors ----
    xT_d = nc.dram_tensor("xT", [128, NR], f8, kind="ExternalInput")
    xoP_d = nc.dram_tensor("x_ownP", [128, NBT * D], bft, kind="ExternalInput")
    Wp_d = nc.dram_tensor("Wp", [128, 144], bft, kind="ExternalInput")
    iota2_d = nc.dram_tensor("iota2", [128, BLK * SLM], bft, kind="ExternalInput")
    I128_d = nc.dram_tensor("I128", [128, 128], bft, kind="ExternalInput")
    W1_d = nc.dram_tensor("W1", [128, 256], bft, kind="ExternalInput")
    W2_d = nc.dram_tensor("W2", [256, 128], bft, kind="ExternalInput")
    b1c_d = nc.dram_tensor("b1col", [128, 2], fp32, kind="ExternalInput")
    gl_d = {}
    if not host["triv_bgat"]:
        gl_d["bgat"] = nc.dram_tensor("bgat_r", [128, 128], fp32, kind="ExternalInput")
    if not host["triv_bff2"]:
        gl_d["bff2"] = nc.dram_tensor("bff2_r", [128, 128], fp32, kind="ExternalInput")
    if not host["triv_gb1"]:
        gl_d["g1"] = nc.dram_tensor("g1_r", [128, 128], fp32, kind="ExternalInput")
        gl_d["b1"] = nc.dram_tensor("b1_r", [128, 128], fp32, kind="ExternalInput")
    if not host["triv_gb2"]:
        gl_d["g2"] = nc.dram_tensor("g2_r", [128, 128], fp32, kind="ExternalInput")
        gl_d["b2"] = nc.dram_tensor("b2_r", [128, 128], fp32, kind="ExternalInput")

    st_d = []
    for sname, Ls, ns in (("lo", L_LO, nsl[0]), ("hi", L_HI, nsl[1])):
        st_d.append({
            "gidx": nc.dram_tensor(f"gidx_{sname}", [128, Ls // 16], i16,
                                   kind="ExternalInput"),
            "dl": nc.dram_tensor(f"dl_{sname}", [128, ns], bft,
                                 kind="ExternalInput"),
            "L": Ls,
        })
    ST4_d = nc.dram_tensor("ST4", [32, host["sg_total"], 128], f8,
                           kind="ExternalInput")
    ad_d = nc.dram_tensor("ad_scratch", [128, NBT * 8], bft, kind="Internal")

    h_d = nc.dram_tensor("h_scratch", [128, NT * 256], bft, kind="Internal")
    z_d = nc.dram_tensor("z", [128, NBT * D], bft, kind="ExternalOutput")

    with tile.TileContext(nc) as tc:
        # ================= consts + preloads =================
        cpool = tc.alloc_tile_pool(name="consts", bufs=1)
        Wp_s = cpool.tile([128, 144], bft)
        nc.sync.dma_start(out=Wp_s[:], in_=Wp_d[:])
        I128_s = cpool.tile([128, 128], bft)
        nc.sync.dma_start(out=I128_s[:], in_=I128_d[:])
        W1_s = cpool.tile([128, 256], bft)
        nc.sync.dma_start(out=W1_s[:], in_=W1_d[:])
        W2_s = cpool.tile([256 // 2, 2, 128], bft)   # [128, 2, 128]
        nc.sync.dma_start(out=W2_s[:],
                          in_=W2_d[:].rearrange("(k h) f -> h k f", k=2))
        b1c_s = cpool.tile([128, 2], fp32)
        nc.sync.dma_start(out=b1c_s[:], in_=b1c_d[:])
        xo_s = cpool.tile([128, NBT * D], bft)
        nc.sync.dma_start(out=xo_s[:], in_=xoP_d[:])
        gl_s = {}
        for k, dref in gl_d.items():
            gl_s[k] = cpool.tile([128, 128], fp32, tag=f"gl_{k}")
            nc.sync.dma_start(out=gl_s[k][:], in_=dref[:])
        iota2_s = cpool.tile([128, BLK, SLM], bft)
        nc.sync.dma_start(out=iota2_s[:],
                          in_=iota2_d[:].rearrange("p (d s) -> p d s", d=BLK))
        eps_s = cpool.tile([128, 1], fp32)
        nc.vector.memset(eps_s[:], LN_EPS)
        dl_s = []
        for s in range(2):
            dlt = cpool.tile([128, nsl[s]], bft, tag=f"dlA{s}")
            nc.sync.dma_start(out=dlt[:], in_=st_d[s]["dl"][:])
            dl_s.append(dlt)

        pD = tc.alloc_tile_pool(name="pD", bufs=1)
        adT32 = pD.tile([32, NBLK, 8], bft)        # a_dst block-major rows

        # ================= phase A =================
        with tc.tile_pool(name="pA", bufs=4) as pA, \
             tc.tile_pool(name="psA", bufs=4, space="PSUM") as psA:
            GT = 3
            XB = 24
            xt = None
            ngrp = (NT + GT - 1) // GT
            for tg in range(ngrp):
                t0 = tg * GT
                ntl = min(GT, NT - t0)
                if t0 % XB == 0:
                    nxb = min(XB, NT - t0)
                    xt = pA.tile([128, XB * 128], f8, tag="xt")
                    nc.sync.dma_start(out=xt[:, :nxb * 128],
                                      in_=xT_d[:, t0 * 128:(t0 + nxb) * 128])
                ps = psA.tile([128, GT, 144], fp32, tag="psA")
                for j in range(ntl):
                    jo = (t0 % XB) + j
                    nc.tensor.matmul(ps[:, j, :],
                                     lhsT=xt[:, jo * 128:(jo + 1) * 128],
                                     rhs=Wp_s[:], start=True, stop=True)
                if tg % 3 == 0:
                    stage = pA.tile([128, 3 * GT, 136], bft, tag="stage")
                if tg % 2 == 0:
                    nc.scalar.activation(
                        out=stage[:, (tg % 3) * GT:(tg % 3) * GT + ntl, :],
                        in_=ps[:, :ntl, 0:136], func=Act.Copy)
                else:
                    nc.vector.tensor_copy(
                        out=stage[:, (tg % 3) * GT:(tg % 3) * GT + ntl, :],
                        in_=ps[:, :ntl, 0:136])
                if t0 < NBT:                       # own a_dst -> DRAM
                    na = min(ntl, NBT - t0)
                    adst = pA.tile([128, GT, 8], bft, tag="adst")
                    nc.vector.tensor_copy(out=adst[:, :na, :],
                                          in_=ps[:, :na, 136:144])
                    nc.scalar.dma_start(
                        out=ad_d[:, t0 * 8:(t0 + na) * 8],
                        in_=adst[:, :na, :])
                s0 = (tg - 2) * GT
                nst = 2 * GT + ntl
                if tg % 3 == 2 or tg == ngrp - 1:
                    if tg % 3 != 2:
                        s0 = (tg - tg % 3) * GT
                        nst = (tg % 3) * GT + ntl
                    nc.scalar.dma_start(
                        out=h_d[:, s0 * 256:(s0 + nst) * 256].rearrange(
                            "p (j d) -> p j d", d=256)[:, :, 0:136],
                        in_=stage[:, :nst, :])

        # ================= phases B + C =================
        nc.sync.dma_start(
            out=adT32[:].rearrange("r (t q) h -> r t q h", q=4),
            in_=ad_d[:].rearrange("(q r) (t h) -> r t q h", q=4, h=8))
        hflat = h_d[:].rearrange("p (t d) -> (p t) d", d=256)
        h_lo = hflat[0:LO_LIM]
        h_hi = hflat[LO_LIM:NR]

        pB = tc.alloc_tile_pool(name="pB", bufs=2)
        pBt = tc.alloc_tile_pool(name="pBtab", bufs=2)
        pBs = tc.alloc_tile_pool(name="pBsmall", bufs=6)
        psB = tc.alloc_tile_pool(name="psB", bufs=3, space="PSUM")
        psE = tc.alloc_tile_pool(name="psE", bufs=2, space="PSUM")
        pC = tc.alloc_tile_pool(name="pC", bufs=2)
        psC = tc.alloc_tile_pool(name="psC", bufs=1, space="PSUM")

        chunk_tiles = [{}, {}]

        def emit_chunk(s, k):
            if k in chunk_tiles[s]:
                return chunk_tiles[s][k]
            sd = st_d[s]
            ngr = min(GPC, sd["L"] // GR - k * GPC)
            ne = ngr * GR
            gix = pBt.tile([128, CHUNK // 16], i16, tag=f"gix{s}")
            nc.sync.dma_start(
                out=gix[:, :ne // 16],
                in_=sd["gidx"][:, k * (CHUNK // 16):k * (CHUNK // 16) + ne // 16])
            ha = pB.tile([128, GPC, 256], bft, tag=f"ha{s}",
                         bufs=3 if s == 0 else 2)
            nc.gpsimd.dma_gather(
                ha[:, :ngr, :], h_lo if s == 0 else h_hi,
                gix[:, :ne // 16],
                ne, ne, 256, single_packet=False)
            # S one-hot [e, d, slot] via is_equal at 2x
            sl0 = chunk_slot0[s][k]
            ns = sum(1 for sl in slots[s] if sl["g"] // GPC == k)
            S_t = pBt.tile([128, BLK, SLM], bft, tag=f"S{s}")
            if ns > 0:
                dv = dl_s[s][:, sl0:sl0 + ns]
                dl_b = AP(dv.tensor, dv.offset,
                          [dv.ap[0], [0, BLK], dv.ap[1]])
                nc.vector.tensor_tensor(out=S_t[:, :, :ns],
                                        in0=iota2_s[:, :, :ns],
                                        in1=dl_b, op=Alu.is_equal)
            # ST4 static fp8 table
            off, nsg = sg_off[(s, k)]
            ST_t = pBt.tile([32, SGM, 128], f8, tag=f"ST{s}")
            if nsg > 0:
                nc.sync.dma_start(out=ST_t[:, :nsg, :],
                                  in_=ST4_d[:, off:off + nsg, :])
            # adE per granule
            adE = psE.tile([128, GPC, 8], fp32, tag="adE")
            for (g_loc, sg, b, st, sp) in ade_sched[(s, k)]:
                nc.tensor.matmul(adE[:, g_loc, :],
                                 lhsT=ST_t[:, sg, :],
                                 rhs=adT32[:, b, :],
                                 start=st, stop=sp)
            # p = exp(leaky(a_src + adE)) over a_src in the gathered tile
            eL = pBs.tile([128, GPC, 8], fp32, tag=f"eL{s}")
            nc.vector.tensor_tensor(out=eL[:, :ngr, :],
                                    in0=ha[:, :ngr, 128:136],
                                    in1=adE[:, :ngr, :], op=Alu.add)
            eL2 = pBs.tile([128, GPC, 8], bft, tag=f"eL2{s}")
            nc.vector.scalar_tensor_tensor(
                out=eL2[:, :ngr, :], in0=eL[:, :ngr, :], scalar=0.2,
                in1=eL[:, :ngr, :], op0=Alu.mult, op1=Alu.max)
            nc.scalar.activation(out=ha[:, :ngr, 128:136], in_=eL2[:, :ngr, :],
                                 func=Act.Exp)
            pv = ha[:, :ngr, 128:136]
            pb = AP(pv.tensor, pv.offset,
                    [pv.ap[0], pv.ap[1], [0, F], pv.ap[2]])
            nc.vector.tensor_tensor(
                out=ha[:, :ngr, 0:128].rearrange("p g (f h) -> p g f h", h=H),
                in0=ha[:, :ngr, 0:128].rearrange("p g (f h) -> p g f h", h=H),
                in1=pb, op=Alu.mult)
            res = {"ha": ha, "S": S_t}
            chunk_tiles[s][k] = res
            return res

        def layer_norm2(tin, g_key, b_key, triv, tagp, npair, out=None,
                        otile=None):
            bst = pBs.tile([128, 2, 6], fp32, tag=f"bst{tagp}")
            for q in range(npair):
                nc.vector.bn_stats(out=bst[:, q, :], in_=tin[:, q, :])
            mv = pBs.tile([128, 2, 2], fp32, tag=f"mv{tagp}")
            for q in range(npair):
                nc.vector.bn_aggr(out=mv[:, q, :], in_=bst[:, q, :])
            # rstd = exp(-0.5 * ln(var + eps)); stays on the exp/ln table
            lnv = pBs.tile([128, 2, 1], fp32, tag=f"lnv{tagp}")
            nc.scalar.activation(out=lnv[:, :npair, :], in_=mv[:, :npair, 1:2],
                                 func=Act.Ln, bias=eps_s[:])
            rstd = pBs.tile([128, 2, 1], fp32, tag=f"rstd{tagp}")
            nc.scalar.activation(out=rstd[:, :npair, :], in_=lnv[:, :npair, :],
                                 func=Act.Exp, scale=-0.5)
            if otile is None:
                otile = pC.tile([128, 2, 128], fp32, tag=f"ln{tagp}")
                out = otile[:]
            for q in range(npair):
                nc.gpsimd.tensor_scalar(out=out[:, q, :], in0=tin[:, q, :],
                                        scalar1=mv[:, q, 0:1],
                                        op0=Alu.subtract,
                                        scalar2=rstd[:, q, :], op1=Alu.mult)
                if not triv:
                    nc.vector.tensor_tensor(out=out[:, q, :], in0=out[:, q, :],
                                            in1=gl_s[g_key][:], op=Alu.mult)
                    nc.vector.tensor_tensor(out=out[:, q, :], in0=out[:, q, :],
                                            in1=gl_s[b_key][:], op=Alu.add)
            return out

        banks = [None, None]
        for t in range(NBT):
            bank = psB.tile([128, 136], fp32, tag="bank")
            for q in range(NCLS):
                b = NCLS * t + q
                for (s, k, sloc, gloc, st, sp) in agg_sched[b]:
                    ct = emit_chunk(s, k)
                    nc.tensor.matmul(bank[BLK * q:BLK * (q + 1), :],
                                     lhsT=ct["S"][:, :, sloc],
                                     rhs=ct["ha"][:, gloc, 0:136],
                                     start=st, stop=sp,
                                     tile_position=(0, BLK * q))
            banks[t % 2] = bank
            if t % 2 == 0 and t != NBT - 1:
                continue
            # ---- phase C for the pair ----
            npair = 1 if t == NBT - 1 and t % 2 == 0 else 2
            t0 = t - npair + 1
            gt = pC.tile([128, 2, 128], fp32, tag="gt")
            rec = pBs.tile([128, 2, 8], fp32, tag="rec")
            for q in range(npair):
                nc.vector.reciprocal(out=rec[:, q, :],
                                     in_=banks[(t0 + q) % 2][:, 128:136])
            for q in range(npair):
                rv = rec[:, q, :]
                rb = AP(rv.tensor, rv.offset, [rv.ap[0], [0, F], rv.ap[1]])
                nc.vector.tensor_tensor(
                    out=gt[:, q, :].rearrange("p (h f) -> p f h", f=F),
                    in0=banks[(t0 + q) % 2][:, 0:128].rearrange(
                        "p (f h) -> p f h", h=H),
                    in1=rb, op=Alu.mult)
                if not host["triv_bgat"]:
                    nc.vector.tensor_tensor(out=gt[:, q, :], in0=gt[:, q, :],
                                            in1=gl_s["bgat"][:], op=Alu.add)
            t1 = pC.tile([128, 2, 128], fp32, tag="t1")
            nc.gpsimd.tensor_tensor(
                out=t1[:, :npair, :],
                in0=xo_s[:, t0 * 128:(t0 + npair) * 128].rearrange(
                    "p (q d) -> p q d", d=128),
                in1=gt[:, :npair, :], op=Alu.add)
            u = layer_norm2(t1[:, :npair, :], "g1", "b1", host["triv_gb1"],
                            "1", npair)
            u_bf = pC.tile([128, 2, 128], bft, tag="ubf")
            nc.scalar.activation(out=u_bf[:, :npair, :], in_=u[:, :npair, :],
                                 func=Act.Copy)
            uT_ps = psC.tile([128, 2, 128], bft, tag="uT")
            for q in range(npair):
                nc.tensor.transpose(uT_ps[:, q, :], in_=u_bf[:, q, :],
                                    identity=I128_s[:])
            uT = pC.tile([128, 2, 128], bft, tag="uTs")
            nc.scalar.activation(out=uT[:, :npair, :], in_=uT_ps[:, :npair, :],
                                 func=Act.Copy)
            f1ps = psC.tile([128, 2, 2, 128], fp32, tag="f1")
            for q in range(npair):
                for j in range(2):
                    nc.tensor.matmul(f1ps[:, q, j, :],
                                     lhsT=W1_s[:, j * 128:(j + 1) * 128],
                                     rhs=uT[:, q, :], start=True, stop=True)
            r1 = pC.tile([128, 2, 2, 128], bft, tag="r1")
            for j in range(2):
                nc.scalar.activation(out=r1[:, :npair, j, :],
                                     in_=f1ps[:, :npair, j, :],
                                     func=Act.Relu, bias=b1c_s[:, j:j + 1])
            zps = psC.tile([128, 2, 128], fp32, tag="zp")
            for q in range(npair):
                for j in range(2):
                    nc.tensor.matmul(zps[:, q, :], lhsT=r1[:, q, j, :],
                                     rhs=W2_s[:, j, :],
                                     start=(j == 0), stop=(j == 1))
            t2 = pC.tile([128, 2, 128], fp32, tag="t2")
            nc.vector.tensor_tensor(out=t2[:, :npair, :], in0=u[:, :npair, :],
                                    in1=zps[:, :npair, :], op=Alu.add)
            if not host["triv_bff2"]:
                for q in range(npair):
                    nc.vector.tensor_tensor(out=t2[:, q, :], in0=t2[:, q, :],
                                            in1=gl_s["bff2"][:], op=Alu.add)
            if t0 % 8 == 0:
                z4 = pC.tile([128, 8, 128], bft, tag="z4")
            layer_norm2(t2[:, :npair, :], "g2", "b2", host["triv_gb2"], "2",
                        npair, out=z4[:, (t0 % 8):(t0 % 8) + npair, :],
                        otile=z4)
            if (t0 + npair) % 8 == 0 or t0 + npair == NBT:
                zb0 = ((t0 + npair - 1) // 8) * 8
                nc.scalar.dma_start(
                    out=z_d[:, zb0 * 128:(t0 + npair) * 128],
                    in_=z4[:, :t0 + npair - zb0, :])

        for p in (psC, pC, psE, psB, pBs, pBt, pB, pD):
            p.release()
        cpool.release()

    nc.compile()
    # insert_act_table_loads indexed tables by position in the (reordered)
    # list we fed it, but walrus interprets act_func_set_id as the index
    # into act_info.json's original order — remap.
    my_names = list(bacc.get_activation_tables(nc.m.arch).keys())
    json_names = list(bacc._orig_gat_tables(nc.m.arch).keys())
    for b in nc.m.functions[0].blocks:
        for i in b.instructions:
            if type(i).__name__ == "InstLoadActFuncSet":
                i.act_func_set_id = json_names.index(my_names[i.act_func_set_id])
    return nc


def kernel(**inputs):
    from concourse.bass_utils import run_bass_kernel_spmd
    import os

    host = _build_host_data(inputs)
    if bool(int(os.environ.get("GAT_DEBUG", "0"))):
        print(f"NT={host['NT']} L_LO={host['L_LO']} L_HI={host['L_HI']} "
              f"slots={len(host['slots'][0])}+{len(host['slots'][1])} "
              f"sg_total={host['sg_total']} SLM={host['SLM']} SGM={host['SGM']} "
              f"pad_edges={host['pad_edges']}")
    nc = _build_program(host)

    in_maps = []
    for c in range(NCORES):
        m = {
            "xT": host["xT"][c],
            "x_ownP": host["x_ownP"][c],
            "Wp": host["Wp"], "I128": host["I128"],
            "iota2": host["iota2"].reshape(128, BLK * host["SLM"]),
            "W1": host["W1"], "W2": host["W2"], "b1col": host["b1col"],
            "ST4": host["per_core"][c]["st4"],
        }
        if not host["triv_bgat"]:
            m["bgat_r"] = np.tile(host["bias_gat"].reshape(1, -1), (128, 1))
        if not host["triv_bff2"]:
            m["bff2_r"] = np.tile(host["b_ff2"].reshape(1, -1), (128, 1))
        if not host["triv_gb1"]:
            m["g1_r"] = np.tile(host["gamma1"].reshape(1, -1), (128, 1))
            m["b1_r"] = np.tile(host["beta1"].reshape(1, -1), (128, 1))
        if not host["triv_gb2"]:
            m["g2_r"] = np.tile(host["gamma2"].reshape(1, -1), (128, 1))
            m["b2_r"] = np.tile(host["beta2"].reshape(1, -1), (128, 1))
        for s, sname in ((0, "lo"), (1, "hi")):
            sd = host["per_core"][c]["streams"][s]
            m[f"gidx_{sname}"] = sd["gidx16"]
            m[f"dl_{sname}"] = sd["dl_bf"]
        in_maps.append(m)

    res = run_bass_kernel_spmd(nc, in_maps, core_ids=list(range(NCORES)))
    if bool(int(os.environ.get("GAT_TIME", "0"))):
        try:
            from concourse.timeline_sim import TimelineSim
            ts = TimelineSim(nc)
            dur = ts.simulate()
            print(f"HW exec time: {dur:.0f} ns (cost-model timeline estimate)")
        except Exception as e:
            print("timeline sim failed:", e)

    out = np.zeros((N, D), np.float32)
    for c in range(NCORES):
        zc = res.results[c]["z"].astype(np.float32)     # [128, NBT*128]
        zc = zc.reshape(128, NBT, D).transpose(1, 0, 2).reshape(OWN, D)
        own = host["own_nodes"][c]
        real = own < N
        out[own[real]] = zc[real]
    return out
